# revision 2
# baseline (speedup 1.0000x reference)
"""MultiHopAttGNN on 8 Trainium2 NeuronCores (Bass/Tile) — single fused NEFF.

Math note (ad-gather elimination): per-edge GAT weight is
w_e = exp(lrelu(as_src + ad_dst, 0.2)) = max(exp(z), exp(0.2 z)).
Softmax alpha = w_e / sum_e w_e is invariant to scaling all weights of a
destination by a constant, so divide by exp(ad_d):
  w'_e(d) = max(exp(as_e), exp(0.2 as_e) * u_d),  u_d = exp(-0.8 ad_d).
as_e rides the gathered table row; u_d depends only on the LOCAL dst tile,
so no per-edge dst gather is needed (halves gather descriptors).

Pipeline (one launch):
  Phase A (per core, nodes sharded by graph id): h = x@W plus as/ad via one
    augmented matmul; writes local table [h | 1 | as] rows (512B-stride,
    fp16) to Internal DRAM; builds per-tile uB broadcast tiles
    [128 x 128] = ones x exp(-0.8 ad_row) held in SBUF.
  AllGather (device collective): local tables -> full [NPAD, 256] table.
  Phase B (per core): per 8192-edge superchunk, dma_gather of [h|1|as] rows
    by src (two halves for int16 idx); A=exp(as), B=exp(0.2 as) on ACT;
    per 128-edge half-chunk V = max(uB*B_e, A_e) and one-hot mask
    (iota == dstl) on DVE/ACT; PE matmul S^T @ [h|1] accumulates segment
    sums + softmax denominators in PSUM; finalize u/s + bias + leaky_relu,
    accumulate 3 hop-sets, mean-pool per graph via host-built (1/count)
    one-hot matmul, FC head + sigmoid. Output [16,1]/core -> [128,1].
"""
import sys
sys.path.insert(0, '/opt/trn_rl_repo')
import inspect
import textwrap
from contextlib import ExitStack

import numpy as np

import concourse.bass as bass
import concourse.bacc as bacc
import concourse.mybir as mybir
import concourse.tile as tile

# Relax dma_gather's elem_size %256 assert (the ucode requires %256 only for
# transpose mode; non-transpose supports arbitrary element sizes).
_src = textwrap.dedent(inspect.getsource(bass.BassGpSimd.dma_gather))
_src = _src.replace("elem_size_bytes > 0 and elem_size_bytes % 256 == 0",
                    "elem_size_bytes > 0")
_ns = vars(bass).copy()
exec(compile(_src, "<dma_gather_patched>", "exec"), _ns)
bass.BassGpSimd.dma_gather = _ns["dma_gather"]

CORES = 8
P = 128
SUPW = 32            # chunks per half-superchunk
D = 128
NG = 128             # graphs
GPC = NG // CORES    # graphs per core
F16 = mybir.dt.float16
F32 = mybir.dt.float32
I16 = mybir.dt.int16
AF = mybir.ActivationFunctionType
OP = mybir.AluOpType


# --------------------------------------------------------------------------
# host-side prep
# --------------------------------------------------------------------------

def wrap_idx(idx_flat):
    n = idx_flat.shape[0]
    base = idx_flat.reshape(n // 16, 16).T.astype(np.int16)
    return np.tile(base, (8, 1))


def prep(inputs):
    out = {}
    batches = [np.asarray(inputs['pro1_batch']), np.asarray(inputs['pro2_batch'])]
    N = batches[0].shape[0]

    bounds = [np.searchsorted(b, np.arange(NG + 1)) for b in batches]
    core_lo = [[int(bounds[br][c * GPC]) for c in range(CORES)] for br in range(2)]
    core_hi = [[int(bounds[br][(c + 1) * GPC]) for c in range(CORES)] for br in range(2)]
    nodes_c = [[core_hi[br][c] - core_lo[br][c] for c in range(CORES)] for br in range(2)]
    NT = max(max((n + P - 1) // P for n in nodes_c[br]) for br in range(2))
    NPC = NT * P
    NPAD = CORES * NPC
    HIBASE = max(0, NPAD - 32000)
    SPLIT = max(HIBASE, (NPAD // 2 // 128) * 128) if NPAD < 32000 else 25600
    assert SPLIT < 32768 and NPAD - HIBASE <= 32000 and HIBASE <= SPLIT
    out.update(NT=NT, NPC=NPC, NPAD=NPAD, SPLIT=SPLIT, HIBASE=HIBASE,
               core_lo=core_lo, core_hi=core_hi)

    gid = []
    for br in range(2):
        g = np.zeros(N, np.int64)
        for c in range(CORES):
            lo, hi = core_lo[br][c], core_hi[br][c]
            g[lo:hi] = c * NPC + (np.arange(lo, hi) - lo)
        gid.append(g)
    out['gid'] = gid

    CH = np.zeros((2, NT, 3), np.int64)
    edges_sorted = [[None] * 3 for _ in range(2)]
    for br in range(2):
        for s in range(3):
            ei = np.asarray(inputs[f'pro{br+1}_ei{s+1}'])
            loops = np.arange(N, dtype=ei.dtype)
            src = np.concatenate([ei[0], loops])
            dst = np.concatenate([ei[1], loops])
            sg = gid[br][src]
            dg = gid[br][dst]
            core_of = dg // NPC
            per_core = []
            for c in range(CORES):
                m = core_of == c
                sc, dc = sg[m], dg[m]
                order = np.argsort(dc, kind='stable')
                sc, dc = sc[order], dc[order]
                dl = dc - c * NPC
                t = dl // P
                ha = sc < SPLIT
                per_core.append((sc, dl, t, ha))
                na = np.bincount(t[ha], minlength=NT)
                nb = np.bincount(t[~ha], minlength=NT)
                need = np.maximum((na + P - 1) // P, (nb + P - 1) // P)
                CH[br, :, s] = np.maximum(CH[br, :, s], need)
            edges_sorted[br][s] = per_core
    out['CH'] = CH

    slot_of_chunk = []
    slot_list = []
    slot_start = []
    pos = 0
    for br in range(2):
        for t in range(NT):
            for s in range(3):
                sid = len(slot_list)
                slot_list.append((br, t, s))
                slot_start.append(pos)
                for _ in range(int(CH[br, t, s])):
                    slot_of_chunk.append(sid)
                    pos += 1
        while pos % SUPW != 0:
            slot_of_chunk.append(-1)
            pos += 1
    L = pos
    NSUP = L // SUPW
    branch_of_sup = []
    for k in range(NSUP):
        sids = [x for x in slot_of_chunk[k * SUPW:(k + 1) * SUPW] if x >= 0]
        branch_of_sup.append(slot_list[sids[0]][0] if sids else 1)
    out.update(slot_of_chunk=slot_of_chunk, slot_list=slot_list, L=L, NSUP=NSUP,
               branch_of_sup=branch_of_sup)

    idxA = np.zeros((CORES, L, P), np.int64)
    idxB = np.zeros((CORES, L, P), np.int64)
    dstl = np.full((CORES, 2, L, P), 999.0, np.float32)
    for c in range(CORES):
        for sid, (br, t, s) in enumerate(slot_list):
            sc, dl, tt, ha = edges_sorted[br][s][c]
            mt = tt == t
            nchunks = int(CH[br, t, s])
            base = slot_start[sid]
            for half in range(2):
                m = mt & (ha if half == 0 else ~ha)
                scm, dlm = sc[m], dl[m]
                if half == 1:
                    scm = scm - HIBASE
                need = scm.shape[0]
                idx_pad = np.zeros(nchunks * P, np.int64)
                idx_pad[:need] = scm
                dl_pad = np.full(nchunks * P, 999.0, np.float32)
                dl_pad[:need] = (dlm - t * P).astype(np.float32)
                tgt = idxA if half == 0 else idxB
                for k in range(nchunks):
                    tgt[c, base + k] = idx_pad[k * P:(k + 1) * P]
                    dstl[c, half, base + k] = dl_pad[k * P:(k + 1) * P]

    ship_idxA = np.zeros((CORES, NSUP, P, SUPW * P // 16), np.int16)
    ship_idxB = np.zeros_like(ship_idxA)
    ship_dstl = np.zeros((CORES, NSUP, P, 2 * SUPW), np.float32)
    for c in range(CORES):
        for k in range(NSUP):
            ship_idxA[c, k] = wrap_idx(idxA[c, k * SUPW:(k + 1) * SUPW].reshape(-1))
            ship_idxB[c, k] = wrap_idx(idxB[c, k * SUPW:(k + 1) * SUPW].reshape(-1))
            ship_dstl[c, k, :, :SUPW] = dstl[c, 0, k * SUPW:(k + 1) * SUPW].T
            ship_dstl[c, k, :, SUPW:] = dstl[c, 1, k * SUPW:(k + 1) * SUPW].T
    out.update(idxA=ship_idxA, idxB=ship_idxB, dstl=ship_dstl)

    pbin = np.zeros((CORES, 2, NT, P, GPC), np.float16)
    for br in range(2):
        cnts = np.bincount(batches[br], minlength=NG).astype(np.float64)
        inv = 1.0 / np.maximum(cnts, 1.0)
        for c in range(CORES):
            lo, hi = core_lo[br][c], core_hi[br][c]
            loc_graph = batches[br][lo:hi] - c * GPC
            loc_node = np.arange(hi - lo)
            pbin[c, br, loc_node // P, loc_node % P, loc_graph] = \
                inv[batches[br][lo:hi]].astype(np.float16)
    out['pbin'] = pbin
    return out


def phase2_plan(pp, act_frac=0.55):
    soc = pp['slot_of_chunk']
    nslots = len(pp['slot_list'])
    first_pos = [None] * nslots
    last_pos = [None] * nslots
    for pos, sid in enumerate(soc):
        if sid < 0:
            continue
        if first_pos[sid] is None:
            first_pos[sid] = pos
        last_pos[sid] = pos
    plan = []
    cnt = 0
    for k in range(pp['NSUP']):
        sup = []
        for j in range(64):
            half = j // 32
            pos = k * SUPW + (j % SUPW)
            sid = soc[pos]
            if sid < 0:
                sup.append((-1, True, True, False, False))
                continue
            start = (half == 0) and (pos == first_pos[sid])
            stop = (half == 1) and (pos == last_pos[sid])
            use_act = (cnt % 100) < int(act_frac * 100)
            cnt += 1
            sup.append((sid, start, stop, stop, use_act))
        plan.append(sup)
    return plan


def build_fused(pp, act_frac=0.55, reps=1, qa=0, qb=0, shared_tbl=True):
    NT, NPC, NPAD, NSUP = pp['NT'], pp['NPC'], pp['NPAD'], pp['NSUP']
    SPLIT, HIBASE = pp['SPLIT'], pp['HIBASE']
    plan = phase2_plan(pp, act_frac)
    slot_list = pp['slot_list']
    bsup = pp['branch_of_sup']
    CH = pp['CH']
    emitted_t = {b: [t for t in range(NT) if CH[b, t].sum() > 0] for b in range(2)}

    nc = bacc.Bacc("TRN2", target_bir_lowering=False, debug=False, num_devices=CORES)
    xT = [nc.dram_tensor(f"xT{b+1}", [1024, NPC], F16, kind="ExternalInput") for b in range(2)]
    Wa = [nc.dram_tensor(f"W{b+1}aug", [1024, 130], F16, kind="ExternalInput") for b in range(2)]
    tloc = [nc.dram_tensor(f"tloc{b+1}", [NPC, 256], F16) for b in range(2)]
    tkw = dict(addr_space="Shared") if shared_tbl else {}
    tbl = [nc.dram_tensor(f"tfull{b+1}", [NPAD, 256], F16, **tkw) for b in range(2)]
    idxA = nc.dram_tensor("idxA", [NSUP, P, 256], I16, kind="ExternalInput")
    idxB = nc.dram_tensor("idxB", [NSUP, P, 256], I16, kind="ExternalInput")
    dstl = nc.dram_tensor("dstl", [NSUP, P, 64], F32, kind="ExternalInput")
    pbin = nc.dram_tensor("pbin", [2, NT, P, GPC], F16, kind="ExternalInput")
    iota = nc.dram_tensor("iota", [P, P], F16, kind="ExternalInput")
    ident = nc.dram_tensor("ident", [P, P], F16, kind="ExternalInput")
    onescol = nc.dram_tensor("onescol", [1, P], F16, kind="ExternalInput")
    brep3 = nc.dram_tensor("brep3", [2, P, 384], F16, kind="ExternalInput")
    pfcW = nc.dram_tensor("pfcW", [2, P, P], F16, kind="ExternalInput")
    pfcb = nc.dram_tensor("pfcb", [2, GPC, P], F16, kind="ExternalInput")
    fc1W = nc.dram_tensor("fc1W", [256, 256], F16, kind="ExternalInput")
    fc1b = nc.dram_tensor("fc1b", [GPC, 256], F16, kind="ExternalInput")
    fc2W = nc.dram_tensor("fc2W", [256, 64], F16, kind="ExternalInput")
    fc2b = nc.dram_tensor("fc2b", [GPC, 64], F16, kind="ExternalInput")
    outW = nc.dram_tensor("outW", [64, 1], F16, kind="ExternalInput")
    outb = nc.dram_tensor("outb", [16, 1], F32, kind="ExternalInput")
    out = nc.dram_tensor("out", [GPC, 1], F32, kind="ExternalOutput")

    with tile.TileContext(nc) as tc:
        with ExitStack() as ctx:
            const = ctx.enter_context(tc.tile_pool(name="const", bufs=1))
            xpool = ctx.enter_context(tc.tile_pool(name="x", bufs=3))
            opool = ctx.enter_context(tc.tile_pool(name="o", bufs=3))
            upool = ctx.enter_context(tc.tile_pool(name="u", bufs=2))
            idxp = ctx.enter_context(tc.tile_pool(name="idx", bufs=3))
            gp = ctx.enter_context(tc.tile_pool(name="g", bufs=3))
            wp = ctx.enter_context(tc.tile_pool(name="wz", bufs=3))
            sp = ctx.enter_context(tc.tile_pool(name="s", bufs=6))
            fin = ctx.enter_context(tc.tile_pool(name="fin", bufs=4))
            ybp = ctx.enter_context(tc.tile_pool(name="yb", bufs=3))
            psum = ctx.enter_context(tc.tile_pool(name="ps", bufs=4, space="PSUM"))
            fcps = ctx.enter_context(tc.tile_pool(name="fcps", bufs=1, space="PSUM"))
            ppool = ctx.enter_context(tc.tile_pool(name="ppool", bufs=1, space="PSUM"))

            # ---------------- constants ----------------
            iota_sb = const.tile([P, P], F16)
            nc.sync.dma_start(iota_sb[:], iota[:, :])
            ident_sb = const.tile([P, P], F16)
            nc.sync.dma_start(ident_sb[:], ident[:, :])
            ones_sb = const.tile([1, P], F16)
            nc.sync.dma_start(ones_sb[:], onescol[:, :])
            brep_sb = const.tile([P, 2, 384], F16)
            for b in range(2):
                nc.sync.dma_start(brep_sb[:, b, :], brep3[b, :, :])
            pbin_sb = const.tile([P, 2, NT, GPC], F16)
            for b in range(2):
                nc.sync.dma_start(pbin_sb[:, b, :, :],
                                  pbin[b].rearrange("t p g -> p t g"))
            pfcW_sb = const.tile([P, 2, P], F16)
            for b in range(2):
                nc.sync.dma_start(pfcW_sb[:, b, :], pfcW[b, :, :])
            pfcb_sb = const.tile([GPC, 2, P], F16)
            for b in range(2):
                nc.sync.dma_start(pfcb_sb[:, b, :], pfcb[b, :, :])
            fc1W_sb = const.tile([P, 2, 256], F16)
            nc.sync.dma_start(fc1W_sb[:, 0, :], fc1W[0:128, :])
            nc.sync.dma_start(fc1W_sb[:, 1, :], fc1W[128:256, :])
            fc1b_sb = const.tile([GPC, 256], F16)
            nc.sync.dma_start(fc1b_sb[:], fc1b[:, :])
            fc2W_sb = const.tile([P, 2, 64], F16)
            nc.sync.dma_start(fc2W_sb[:, 0, :], fc2W[0:128, :])
            nc.sync.dma_start(fc2W_sb[:, 1, :], fc2W[128:256, :])
            fc2b_sb = const.tile([GPC, 64], F16)
            nc.sync.dma_start(fc2b_sb[:], fc2b[:, :])
            outW_sb = const.tile([64, 1], F16)
            nc.sync.dma_start(outW_sb[:], outW[:, :])
            outb_sb = const.tile([GPC, 1], F32)
            nc.sync.dma_start(outb_sb[:], outb[:, :])

            wt = []
            for b in range(2):
                w = const.tile([P, 8, 130], F16, tag=f"w{b}", name=f"w{b}")
                nc.sync.dma_start(w[:, :, :], Wa[b].rearrange("(k p) j -> p k j", p=P))
                wt.append(w)

            # uB broadcast tiles live in SBUF across the whole kernel
            uB_all = const.tile([P, 2, NT, P], F16, tag="uB", name="uB_all")

            for rep in range(reps):
              # ---------------- phase A ----------------
              for b in range(2):
                for t in range(NT):
                    xt = xpool.tile([P, 8, P], F16, tag="xt", name=f"xt{rep}_{b}_{t}")
                    nc.sync.dma_start(
                        xt[:, :, :],
                        xT[b][:, t * P:(t + 1) * P].rearrange("(k p) j -> p k j", p=P))
                    ps = psum.tile([P, 130], F32, tag="agg", name=f"p1ps{rep}_{b}_{t}")
                    for k in range(8):
                        nc.tensor.matmul(out=ps[:], lhsT=xt[:, k, :], rhs=wt[b][:, k, :],
                                         start=(k == 0), stop=(k == 7))
                    sb = opool.tile([P, 130], F16, tag="sb", name=f"sb{rep}_{b}_{t}")
                    nc.vector.tensor_copy(sb[:, 0:128], ps[:, 0:128])
                    nc.vector.tensor_copy(sb[:, 129:130], ps[:, 128:129])
                    nc.vector.memset(sb[:, 128:129], 1.0)
                    nc.sync.dma_start(tloc[b][t * P:(t + 1) * P, 0:130], sb[:])
                    # u column: exp(-0.8 * ad)
                    uc = opool.tile([P, 1], F16, tag="uc", name=f"uc{rep}_{b}_{t}")
                    nc.scalar.activation(uc[:], ps[:, 129:130], AF.Exp, scale=-0.8)
                    urow_ps = fcps.tile([1, P], F16, tag="fcT", name=f"urps{rep}_{b}_{t}")
                    nc.tensor.transpose(out=urow_ps[:], in_=uc[:], identity=ident_sb[:, :])
                    urow = upool.tile([1, P], F16, tag="ur", name=f"ur{rep}_{b}_{t}")
                    nc.vector.tensor_copy(urow[:], urow_ps[:])
                    ub_ps = fcps.tile([P, P], F32, tag="fc", name=f"ubps{rep}_{b}_{t}")
                    nc.tensor.matmul(out=ub_ps[:], lhsT=ones_sb[:], rhs=urow[:],
                                     start=True, stop=True)
                    nc.vector.tensor_copy(uB_all[:, b, t, :], ub_ps[:])

              # ---------------- AllGather ----------------
              for b in range(2):
                nc.gpsimd.collective_compute(
                    "AllGather", OP.bypass,
                    replica_groups=[list(range(CORES))],
                    ins=[tloc[b][:, :]], outs=[tbl[b][:, :]],
                )

              # ---------------- phase B ----------------
              poolps = []
              for b in range(2):
                pool_t = ppool.tile([P, GPC], F32, tag=f"pool{b}", name=f"poolps{rep}_{b}")
                poolps.append(pool_t)

              live_ps = {}
              live_yb = {}

              def finalize(sid, rep=rep, poolps=poolps, live_ps=live_ps, live_yb=live_yb):
                br, t, s = slot_list[sid]
                ps = live_ps.pop(sid)
                s_sb = fin.tile([P, 1], F32, tag="ssb", name=f"ssb{rep}_{sid}")
                nc.vector.tensor_scalar(out=s_sb[:], in0=ps[:, 128:129],
                                        scalar1=1e-12, scalar2=None, op0=OP.max)
                r_sb = fin.tile([P, 1], F32, tag="rsb", name=f"rsb{rep}_{sid}")
                nc.vector.reciprocal(r_sb[:], s_sb[:])
                if s == 0:
                    live_yb[(br, t)] = ybp.tile([P, 384], F16, tag="ybuf", name=f"ybuf{rep}_{br}_{t}")
                yb = live_yb[(br, t)]
                nc.vector.tensor_scalar(out=yb[:, s * 128:(s + 1) * 128],
                                        in0=ps[:, 0:128],
                                        scalar1=r_sb[:, 0:1], scalar2=None,
                                        op0=OP.mult)
                if s == 2:
                    live_yb.pop((br, t))
                    yb2 = fin.tile([P, 384], F16, tag="yb2", name=f"yb2_{rep}_{sid}")
                    nc.vector.tensor_tensor(out=yb2[:], in0=yb[:], in1=brep_sb[:, br, :], op=OP.add)
                    t2 = fin.tile([P, 384], F16, tag="t2", name=f"t2_{rep}_{sid}")
                    nc.vector.tensor_scalar(out=t2[:], in0=yb2[:], scalar1=0.01,
                                            scalar2=None, op0=OP.mult)
                    m = fin.tile([P, 384], F16, tag="m", name=f"m_{rep}_{sid}")
                    nc.vector.tensor_tensor(out=m[:], in0=yb2[:], in1=t2[:], op=OP.max)
                    hs = fin.tile([P, 128], F16, tag="hs", name=f"hs_{rep}_{sid}")
                    nc.vector.tensor_tensor(out=hs[:], in0=m[:, 0:128], in1=m[:, 128:256], op=OP.add)
                    nc.vector.tensor_tensor(out=hs[:], in0=hs[:], in1=m[:, 256:384], op=OP.add)
                    nc.tensor.matmul(out=poolps[br][:], lhsT=hs[:],
                                     rhs=pbin_sb[:, br, t, :],
                                     start=(t == emitted_t[br][0]),
                                     stop=(t == emitted_t[br][-1]))

              for k in range(NSUP):
                br = bsup[k]
                ia = idxp.tile([P, 256], I16, tag="ia", name=f"ia{rep}_{k}")
                nc.sync.dma_start(ia[:], idxA[k, :, :])
                ib = idxp.tile([P, 256], I16, tag="ib", name=f"ib{rep}_{k}")
                nc.sync.dma_start(ib[:], idxB[k, :, :])
                dl = idxp.tile([P, 64], F32, tag="dl", name=f"dl{rep}_{k}")
                nc.sync.dma_start(dl[:], dstl[k, :, :])

                g = gp.tile([P, 64, 130], F16, tag="g", name=f"g{rep}_{k}")
                nc.gpsimd.dma_gather(
                    out_ap=g[:, 0:32, :], in_ap=tbl[br][0:SPLIT, 0:130],
                    idxs_ap=ia[:, :], num_idxs=4096, num_idxs_reg=4096,
                    elem_size=130, elem_step=256, single_packet=False,
                    queue_num=qa)
                nc.gpsimd.dma_gather(
                    out_ap=g[:, 32:64, :], in_ap=tbl[br][HIBASE:NPAD, 0:130],
                    idxs_ap=ib[:, :], num_idxs=4096, num_idxs_reg=4096,
                    elem_size=130, elem_step=256, single_packet=False,
                    queue_num=qb)

                # A = exp(as), B = exp(0.2 as) from the gathered as column
                A_t = wp.tile([P, 64], F32, tag="A", name=f"A{rep}_{k}")
                nc.scalar.activation(A_t[:], g[:, :, 129], AF.Exp)
                B_t = wp.tile([P, 64], F32, tag="B", name=f"B{rep}_{k}")
                nc.scalar.activation(B_t[:], g[:, :, 129], AF.Exp, scale=0.2)
                negd = wp.tile([P, 64], F32, tag="negd", name=f"negd{rep}_{k}")
                nc.vector.tensor_scalar(out=negd[:], in0=dl[:], scalar1=-1.0, scalar2=None, op0=OP.mult)

                for p_pos in range(32):
                  for half in range(2):
                    j = half * 32 + p_pos
                    sid, start, stop, do_fin, use_act = plan[k][j]
                    if sid < 0:
                        continue
                    br_, t_, s_ = slot_list[sid]
                    if start:
                        live_ps[sid] = psum.tile([P, 129], F32, tag="agg", name=f"aggps{rep}_{sid}")
                    ps = live_ps[sid]
                    V = sp.tile([P, P], F16, tag="V", name=f"V{rep}_{k}_{j}")
                    nc.vector.tensor_scalar(out=V[:], in0=uB_all[:, br_, t_, :],
                                            scalar1=B_t[:, j:j + 1],
                                            scalar2=A_t[:, j:j + 1],
                                            op0=OP.mult, op1=OP.max)
                    S = sp.tile([P, P], F16, tag="S", name=f"S{rep}_{k}_{j}")
                    if use_act:
                        a_t = sp.tile([P, P], F16, tag="a", name=f"a{rep}_{k}_{j}")
                        nc.scalar.activation(a_t[:], iota_sb[:], AF.Abs,
                                             bias=negd[:, j:j + 1])
                        o_t = sp.tile([P, P], F16, tag="oh", name=f"oh{rep}_{k}_{j}")
                        nc.scalar.activation(o_t[:], a_t[:], AF.Relu,
                                             bias=1.0, scale=-1.0)
                    else:
                        o_t = sp.tile([P, P], F16, tag="oh", name=f"oh{rep}_{k}_{j}")
                        nc.vector.tensor_scalar(out=o_t[:], in0=iota_sb[:],
                                                scalar1=dl[:, j:j + 1],
                                                scalar2=None, op0=OP.is_equal)
                    nc.vector.tensor_tensor(out=S[:], in0=o_t[:], in1=V[:], op=OP.mult)
                    nc.tensor.matmul(out=ps[:], lhsT=S[:], rhs=g[:, j, 0:129],
                                     start=start, stop=stop)
                    if do_fin:
                        finalize(sid)

              # ---------------- FC head ----------------
              xT_sb = []
              for b in range(2):
                pT = fin.tile([P, GPC], F16, tag="pT", name=f"pT{rep}_{b}")
                nc.vector.tensor_copy(pT[:], poolps[b][:])
                ps1 = fcps.tile([GPC, P], F32, tag="fc", name=f"ps1_{rep}_{b}")
                nc.tensor.matmul(out=ps1[:], lhsT=pT[:], rhs=pfcW_sb[:, b, :], start=True, stop=True)
                xb = fin.tile([GPC, P], F16, tag="xb", name=f"xb{rep}_{b}")
                nc.vector.tensor_tensor(out=xb[:], in0=ps1[:], in1=pfcb_sb[:, b, :], op=OP.add)
                t2 = fin.tile([GPC, P], F16, tag="xbt", name=f"xbt{rep}_{b}")
                nc.vector.tensor_scalar(out=t2[:], in0=xb[:], scalar1=0.01, scalar2=None, op0=OP.mult)
                nc.vector.tensor_tensor(out=xb[:], in0=xb[:], in1=t2[:], op=OP.max)
                psT = fcps.tile([P, GPC], F16, tag="fcT", name=f"psT{rep}_{b}")
                nc.tensor.transpose(out=psT[:], in_=xb[:], identity=ident_sb[0:GPC, 0:GPC])
                xTs = fin.tile([P, GPC], F16, tag=f"xT{b}", name=f"xTs{rep}_{b}")
                nc.vector.tensor_copy(xTs[:], psT[:])
                xT_sb.append(xTs)

              ps2 = fcps.tile([GPC, 256], F32, tag="fc", name=f"ps2_{rep}")
              nc.tensor.matmul(out=ps2[:], lhsT=xT_sb[0][:], rhs=fc1W_sb[:, 0, :], start=True, stop=False)
              nc.tensor.matmul(out=ps2[:], lhsT=xT_sb[1][:], rhs=fc1W_sb[:, 1, :], start=False, stop=True)
              y1 = fin.tile([GPC, 256], F16, tag="y1", name=f"y1_{rep}")
              nc.vector.tensor_tensor(out=y1[:], in0=ps2[:], in1=fc1b_sb[:], op=OP.add)
              t2 = fin.tile([GPC, 256], F16, tag="y1t", name=f"y1t_{rep}")
              nc.vector.tensor_scalar(out=t2[:], in0=y1[:], scalar1=0.01, scalar2=None, op0=OP.mult)
              nc.vector.tensor_tensor(out=y1[:], in0=y1[:], in1=t2[:], op=OP.max)

              y1T = []
              for hlf in range(2):
                psT = fcps.tile([P, GPC], F16, tag="fcT", name=f"psTy{rep}_{hlf}")
                nc.tensor.transpose(out=psT[:], in_=y1[:, hlf * 128:(hlf + 1) * 128],
                                    identity=ident_sb[0:GPC, 0:GPC])
                yt = fin.tile([P, GPC], F16, tag=f"y1T{hlf}", name=f"y1T{rep}_{hlf}")
                nc.vector.tensor_copy(yt[:], psT[:])
                y1T.append(yt)

              ps3 = fcps.tile([GPC, 64], F32, tag="fc", name=f"ps3_{rep}")
              nc.tensor.matmul(out=ps3[:], lhsT=y1T[0][:], rhs=fc2W_sb[:, 0, :], start=True, stop=False)
              nc.tensor.matmul(out=ps3[:], lhsT=y1T[1][:], rhs=fc2W_sb[:, 1, :], start=False, stop=True)
              y2 = fin.tile([GPC, 64], F16, tag="y2", name=f"y2_{rep}")
              nc.vector.tensor_tensor(out=y2[:], in0=ps3[:], in1=fc2b_sb[:], op=OP.add)
              t2 = fin.tile([GPC, 64], F16, tag="y2t", name=f"y2t_{rep}")
              nc.vector.tensor_scalar(out=t2[:], in0=y2[:], scalar1=0.01, scalar2=None, op0=OP.mult)
              nc.vector.tensor_tensor(out=y2[:], in0=y2[:], in1=t2[:], op=OP.max)

              psT = fcps.tile([64, GPC], F16, tag="fcT", name=f"psTy2_{rep}")
              nc.tensor.transpose(out=psT[:], in_=y2[:], identity=ident_sb[0:GPC, 0:GPC])
              y2T = fin.tile([64, GPC], F16, tag="y2T", name=f"y2T_{rep}")
              nc.vector.tensor_copy(y2T[:], psT[:])

              ps4 = fcps.tile([GPC, 1], F32, tag="fc", name=f"ps4_{rep}")
              nc.tensor.matmul(out=ps4[:], lhsT=y2T[:], rhs=outW_sb[:], start=True, stop=True)
              o_sb = fin.tile([GPC, 1], F32, tag="o", name=f"o_sb{rep}")
              nc.scalar.activation(o_sb[:], ps4[:], AF.Sigmoid, bias=outb_sb[:, 0:1])
              nc.sync.dma_start(out[:, :], o_sb[:])
    nc.compile()
    return nc


# --------------------------------------------------------------------------
# host-side input assembly
# --------------------------------------------------------------------------

def host_inputs(inputs, pp):
    NPC = pp['NPC']
    wa = []
    for b in range(2):
        W = np.asarray(inputs[f'W{b+1}'], np.float64)
        a_s = np.asarray(inputs[f'a{b+1}_src'], np.float64)
        a_d = np.asarray(inputs[f'a{b+1}_dst'], np.float64)
        w_aug = np.concatenate([W, (W @ a_s)[:, None], (W @ a_d)[:, None]], axis=1)
        wa.append(w_aug.astype(np.float16))
    xs = [np.asarray(inputs['pro1_x']), np.asarray(inputs['pro2_x'])]

    iota = np.tile(np.arange(P, dtype=np.float16)[None, :], (P, 1))
    ident = np.eye(P, dtype=np.float16)
    onescol = np.ones((1, P), np.float16)
    brep3 = np.zeros((2, P, 384), np.float16)
    for b in range(2):
        bb = np.asarray(inputs[f'b{b+1}'], np.float32).astype(np.float16)
        brep3[b] = np.tile(bb[None, :], (P, 3))
    pfcW = np.stack([np.asarray(inputs['p1fc_W']), np.asarray(inputs['p2fc_W'])]).astype(np.float16)
    pfcb = np.stack([
        np.tile(np.asarray(inputs['p1fc_b'])[None, :], (GPC, 1)),
        np.tile(np.asarray(inputs['p2fc_b'])[None, :], (GPC, 1)),
    ]).astype(np.float16)
    fc1W = np.asarray(inputs['fc1_W']).astype(np.float16)
    fc1b = np.tile(np.asarray(inputs['fc1_b'])[None, :], (GPC, 1)).astype(np.float16)
    fc2W = np.asarray(inputs['fc2_W']).astype(np.float16)
    fc2b = np.tile(np.asarray(inputs['fc2_b'])[None, :], (GPC, 1)).astype(np.float16)
    outW = np.asarray(inputs['out_W']).astype(np.float16)
    outb = np.tile(np.asarray(inputs['out_b']).reshape(1, 1), (GPC, 1)).astype(np.float32)

    maps = []
    for c in range(CORES):
        m = dict(
            idxA=pp['idxA'][c], idxB=pp['idxB'][c],
            dstl=pp['dstl'][c], pbin=pp['pbin'][c],
            iota=iota, ident=ident, onescol=onescol, brep3=brep3,
            pfcW=pfcW, pfcb=pfcb, fc1W=fc1W, fc1b=fc1b,
            fc2W=fc2W, fc2b=fc2b, outW=outW, outb=outb,
        )
        for b in range(2):
            lo, hi = pp['core_lo'][b][c], pp['core_hi'][b][c]
            xt = np.zeros((1024, NPC), np.float16)
            xt[:, 0:hi - lo] = xs[b][lo:hi].T.astype(np.float16)
            m[f'xT{b+1}'] = xt
            m[f'W{b+1}aug'] = wa[b]
        maps.append(m)
    return maps


_CACHE = {}


def _cache_key(pp):
    return (pp['NT'], pp['NSUP'], tuple(pp['branch_of_sup']),
            tuple(int(x) for x in pp['CH'].reshape(-1)))


def get_nc(pp, **kw):
    key = (_cache_key(pp), tuple(sorted(kw.items())))
    if key not in _CACHE:
        _CACHE[key] = build_fused(pp, **kw)
    return _CACHE[key]


def kernel(**inputs):
    from concourse import bass_utils
    inputs = {k: np.asarray(v) for k, v in inputs.items()}
    pp = prep(inputs)
    nc = get_nc(pp)
    maps = host_inputs(inputs, pp)
    res = bass_utils.run_bass_kernel_spmd(nc, maps, core_ids=list(range(CORES)))
    return np.concatenate([res.results[c]['out'] for c in range(CORES)], axis=0)


# revision 3
# speedup vs baseline: 1.8298x; 1.8298x over previous
"""MultiHopAttGNN on 8 Trainium2 NeuronCores — fused NEFF, v4.

v4 over v3 (kernel_fused):
  - Per-half chunk counts decoupled: chunks(slot, half) = max over cores of
    ceil(n_edges/128) per half independently (was max over both halves),
    and superchunks take a variable A/B split -> ~10% fewer gather
    descriptors (the gather is HBM-latency-bound at ~283 ns/desc/engine).
  - Edges within each (slot, half) sorted by src index for HBM row
    locality; pad lanes duplicate the previous real src row (row hit).
  - Partial last gathers use exact compile-time num_idxs.
See kernel_fused.py docstring for the pipeline and the ad-gather
elimination math.
"""
import sys
sys.path.insert(0, '/opt/trn_rl_repo')
import inspect
import textwrap
from contextlib import ExitStack

import numpy as np

import concourse.bass as bass
import concourse.bacc as bacc
import concourse.mybir as mybir
import concourse.tile as tile

_src = textwrap.dedent(inspect.getsource(bass.BassGpSimd.dma_gather))
_src = _src.replace("elem_size_bytes > 0 and elem_size_bytes % 256 == 0",
                    "elem_size_bytes > 0")
_ns = vars(bass).copy()
exec(compile(_src, "<dma_gather_patched>", "exec"), _ns)
bass.BassGpSimd.dma_gather = _ns["dma_gather"]

CORES = 8
P = 128
SUPC = 64            # chunks per superchunk
D = 128
NG = 128
GPC = NG // CORES
F16 = mybir.dt.float16
F32 = mybir.dt.float32
I16 = mybir.dt.int16
AF = mybir.ActivationFunctionType
OP = mybir.AluOpType


def wrap_idx(idx_flat):
    n = idx_flat.shape[0]
    base = idx_flat.reshape(n // 16, 16).T.astype(np.int16)
    return np.tile(base, (8, 1))


def prep(inputs):
    out = {}
    batches = [np.asarray(inputs['pro1_batch']), np.asarray(inputs['pro2_batch'])]
    N = batches[0].shape[0]

    bounds = [np.searchsorted(b, np.arange(NG + 1)) for b in batches]
    core_lo = [[int(bounds[br][c * GPC]) for c in range(CORES)] for br in range(2)]
    core_hi = [[int(bounds[br][(c + 1) * GPC]) for c in range(CORES)] for br in range(2)]
    nodes_c = [[core_hi[br][c] - core_lo[br][c] for c in range(CORES)] for br in range(2)]
    NT = max(max((n + P - 1) // P for n in nodes_c[br]) for br in range(2))
    NPC = NT * P
    NPAD = CORES * NPC
    HIBASE = max(0, NPAD - 32000)
    SPLIT = max(HIBASE, (NPAD // 2 // 128) * 128) if NPAD < 32000 else 25600
    assert SPLIT < 32768 and NPAD - HIBASE <= 32000 and HIBASE <= SPLIT
    out.update(NT=NT, NPC=NPC, NPAD=NPAD, SPLIT=SPLIT, HIBASE=HIBASE,
               core_lo=core_lo, core_hi=core_hi)

    gid = []
    for br in range(2):
        g = np.zeros(N, np.int64)
        for c in range(CORES):
            lo, hi = core_lo[br][c], core_hi[br][c]
            g[lo:hi] = c * NPC + (np.arange(lo, hi) - lo)
        gid.append(g)

    # per-(branch, tile, set, half) edge lists per core, sorted by src
    CHh = np.zeros((2, NT, 3, 2), np.int64)
    ed = {}   # (br,t,s,h,c) -> (src_idx_for_half, dstl_local)
    for br in range(2):
        for s in range(3):
            ei = np.asarray(inputs[f'pro{br+1}_ei{s+1}'])
            loops = np.arange(N, dtype=ei.dtype)
            src = np.concatenate([ei[0], loops])
            dst = np.concatenate([ei[1], loops])
            sg = gid[br][src]
            dg = gid[br][dst]
            core_of = dg // NPC
            for c in range(CORES):
                m = core_of == c
                sc, dc = sg[m], dg[m]
                dl = dc - c * NPC
                t_of = dl // P
                ha = sc < SPLIT
                for h in range(2):
                    mh = ha if h == 0 else ~ha
                    sch = sc[mh] if h == 0 else sc[mh] - HIBASE
                    dlh = dl[mh]
                    th = t_of[mh]
                    for t in range(NT):
                        mt = th == t
                        scm, dlm = sch[mt], dlh[mt]
                        order = np.argsort(scm, kind='stable')
                        scm, dlm = scm[order], dlm[order]
                        ed[(br, t, s, h, c)] = (scm, dlm - t * P)
                        CHh[br, t, s, h] = max(CHh[br, t, s, h],
                                               (scm.shape[0] + P - 1) // P)
    out['CHh'] = CHh

    # chunk lists per branch (A-half list and B-half list, in (t,s) order)
    slot_list = []          # sid -> (br, t, s)
    sid_of = {}
    for br in range(2):
        for t in range(NT):
            for s in range(3):
                sid_of[(br, t, s)] = len(slot_list)
                slot_list.append((br, t, s))
    out['slot_list'] = slot_list

    half_lists = {0: ([], []), 1: ([], [])}   # br -> (Alist, Blist)
    for br in range(2):
        for t in range(NT):
            for s in range(3):
                for h in range(2):
                    lst = half_lists[br][h]
                    for cseq in range(int(CHh[br, t, s, h])):
                        lst.append((sid_of[(br, t, s)], h, cseq))

    # assign chunks to superchunks as groups of whole slots (a slot's A and B
    # chunks land in one superchunk -> few live PSUM accumulators). Partial
    # superchunks cost nothing: gathers use exact chunk counts.
    sup_meta = []    # per superchunk: dict(br, nA, nB, chunks=[(sid,h,cseq)...])
    for br in range(2):
        # per-slot (A-chunks, B-chunks) in sid order
        slots_br = {}
        order = []
        for h in range(2):
            for item in half_lists[br][h]:
                sid = item[0]
                if sid not in slots_br:
                    slots_br[sid] = ([], [])
                    order.append(sid)
                slots_br[sid][h].append(item)
        order.sort()
        group_a, group_b = [], []
        def emit(group_a=None, group_b=None, _sup=sup_meta, _br=br):
            pass
        i = 0
        while i < len(order):
            sid = order[i]
            ach, bch = slots_br[sid]
            n_new = len(ach) + len(bch)
            cur = len(group_a) + len(group_b)
            if cur > 0 and cur + n_new > SUPC:
                chunks = group_a + group_b
                nA, nB = len(group_a), len(group_b)
                chunks += [None] * (SUPC - len(chunks))
                sup_meta.append(dict(br=br, nA=nA, nB=nB, chunks=chunks))
                group_a, group_b = [], []
                continue
            if n_new > SUPC:
                # pathological giant slot: split across superchunks
                rem_a, rem_b = list(ach), list(bch)
                while rem_a or rem_b:
                    room = SUPC - len(group_a) - len(group_b)
                    ta = rem_a[:room]
                    rem_a = rem_a[len(ta):]
                    room -= len(ta)
                    tb = rem_b[:room]
                    rem_b = rem_b[len(tb):]
                    group_a.extend(ta)
                    group_b.extend(tb)
                    if len(group_a) + len(group_b) == SUPC:
                        chunks = group_a + group_b
                        sup_meta.append(dict(br=br, nA=len(group_a), nB=len(group_b), chunks=chunks))
                        group_a, group_b = [], []
                i += 1
                continue
            group_a.extend(ach)
            group_b.extend(bch)
            i += 1
        if group_a or group_b:
            chunks = group_a + group_b
            nA, nB = len(group_a), len(group_b)
            chunks += [None] * (SUPC - len(chunks))
            sup_meta.append(dict(br=br, nA=nA, nB=nB, chunks=chunks))
    NSUP = len(sup_meta)
    out.update(NSUP=NSUP, sup_meta=sup_meta)

    # start/stop per chunk in emission order; max live psum
    nslots = len(slot_list)
    first_pos = [None] * nslots
    last_pos = [None] * nslots
    pos = 0
    pos_of = {}
    for k, sm in enumerate(sup_meta):
        for j in range(SUPC):
            ch = sm['chunks'][j]
            if ch is None:
                pos += 1
                continue
            sid = ch[0]
            if first_pos[sid] is None:
                first_pos[sid] = pos
            last_pos[sid] = pos
            pos_of[(k, j)] = pos
            pos += 1
    live = 0
    max_live = 0
    pos = 0
    for k, sm in enumerate(sup_meta):
        for j in range(SUPC):
            ch = sm['chunks'][j]
            if ch is not None:
                sid = ch[0]
                if pos == first_pos[sid]:
                    live += 1
                    max_live = max(max_live, live)
                if pos == last_pos[sid]:
                    live -= 1
            pos += 1
    out['max_live'] = max_live

    plan = []
    pos = 0
    cnt = 0
    act_frac = 0.55
    for k, sm in enumerate(sup_meta):
        sup = []
        for j in range(SUPC):
            ch = sm['chunks'][j]
            if ch is None:
                sup.append((-1, False, False, False))
                pos += 1
                continue
            sid = ch[0]
            start = pos == first_pos[sid]
            stop = pos == last_pos[sid]
            use_act = (cnt % 100) < int(act_frac * 100)
            cnt += 1
            sup.append((sid, start, stop, use_act))
            pos += 1
        plan.append(sup)
    out['plan'] = plan

    # per-core idx / dstl tables
    ship_idxA = np.zeros((CORES, NSUP, P, 512), np.int16)
    ship_idxB = np.zeros((CORES, NSUP, P, 512), np.int16)
    ship_dstl = np.full((CORES, NSUP, P, SUPC), 999.0, np.float32)
    for c in range(CORES):
        # pad each (sid,h) to CHh*128 with dup-last idx and dstl=999
        padded = {}
        for (br, t, s, h, cc), (scm, dlm) in ed.items():
            if cc != c:
                continue
            nch = int(CHh[br, t, s, h])
            idx_pad = np.zeros(nch * P, np.int64)
            dl_pad = np.full(nch * P, 999.0, np.float32)
            n = scm.shape[0]
            idx_pad[:n] = scm
            if n > 0 and n < nch * P:
                idx_pad[n:] = scm[-1]
            dl_pad[:n] = dlm
            padded[(sid_of[(br, t, s)], h)] = (idx_pad, dl_pad)
        for k, sm in enumerate(sup_meta):
            nA, nB = sm['nA'], sm['nB']
            a_idx = np.zeros(SUPC * P // 2 * 2, np.int64)  # scratch
            aflat = []
            bflat = []
            for j in range(SUPC):
                ch = sm['chunks'][j]
                if ch is None:
                    continue
                sid, h, cseq = ch
                idx_pad, dl_pad = padded[(sid, h)]
                seg_i = idx_pad[cseq * P:(cseq + 1) * P]
                seg_d = dl_pad[cseq * P:(cseq + 1) * P]
                (aflat if h == 0 else bflat).append(seg_i)
                ship_dstl[c, k, :, j] = seg_d
            if aflat:
                fa = np.concatenate(aflat)
                ship_idxA[c, k, :, 0:fa.shape[0] // 16] = wrap_idx(fa)[:, 0:fa.shape[0] // 16]
            if bflat:
                fb = np.concatenate(bflat)
                ship_idxB[c, k, :, 0:fb.shape[0] // 16] = wrap_idx(fb)[:, 0:fb.shape[0] // 16]
    out.update(idxA=ship_idxA, idxB=ship_idxB, dstl=ship_dstl)

    pbin = np.zeros((CORES, 2, NT, P, GPC), np.float16)
    for br in range(2):
        cnts = np.bincount(batches[br], minlength=NG).astype(np.float64)
        inv = 1.0 / np.maximum(cnts, 1.0)
        for c in range(CORES):
            lo, hi = core_lo[br][c], core_hi[br][c]
            loc_graph = batches[br][lo:hi] - c * GPC
            loc_node = np.arange(hi - lo)
            pbin[c, br, loc_node // P, loc_node % P, loc_graph] = \
                inv[batches[br][lo:hi]].astype(np.float16)
    out['pbin'] = pbin
    return out


def build_fused(pp, reps=1, shared_tbl=True, psum_bufs=None):
    NT, NPC, NPAD, NSUP = pp['NT'], pp['NPC'], pp['NPAD'], pp['NSUP']
    SPLIT, HIBASE = pp['SPLIT'], pp['HIBASE']
    plan = pp['plan']
    slot_list = pp['slot_list']
    sup_meta = pp['sup_meta']
    CHh = pp['CHh']
    emitted_t = {b: [t for t in range(NT) if CHh[b, t].sum() > 0] for b in range(2)}
    if psum_bufs is None:
        psum_bufs = 5
    assert pp['max_live'] <= psum_bufs

    nc = bacc.Bacc("TRN2", target_bir_lowering=False, debug=False, num_devices=CORES)
    xT = [nc.dram_tensor(f"xT{b+1}", [1024, NPC], F16, kind="ExternalInput") for b in range(2)]
    Wa = [nc.dram_tensor(f"W{b+1}aug", [1024, 130], F16, kind="ExternalInput") for b in range(2)]
    tloc = [nc.dram_tensor(f"tloc{b+1}", [NPC, 256], F16) for b in range(2)]
    tkw = dict(addr_space="Shared") if shared_tbl else {}
    tbl = [nc.dram_tensor(f"tfull{b+1}", [NPAD, 256], F16, **tkw) for b in range(2)]
    idxA = nc.dram_tensor("idxA", [NSUP, P, 512], I16, kind="ExternalInput")
    idxB = nc.dram_tensor("idxB", [NSUP, P, 512], I16, kind="ExternalInput")
    dstl = nc.dram_tensor("dstl", [NSUP, P, SUPC], F32, kind="ExternalInput")
    pbin = nc.dram_tensor("pbin", [2, NT, P, GPC], F16, kind="ExternalInput")
    iota = nc.dram_tensor("iota", [P, P], F16, kind="ExternalInput")
    ident = nc.dram_tensor("ident", [P, P], F16, kind="ExternalInput")
    onescol = nc.dram_tensor("onescol", [1, P], F16, kind="ExternalInput")
    brep3 = nc.dram_tensor("brep3", [2, P, 384], F16, kind="ExternalInput")
    pfcW = nc.dram_tensor("pfcW", [2, P, P], F16, kind="ExternalInput")
    pfcb = nc.dram_tensor("pfcb", [2, GPC, P], F16, kind="ExternalInput")
    fc1W = nc.dram_tensor("fc1W", [256, 256], F16, kind="ExternalInput")
    fc1b = nc.dram_tensor("fc1b", [GPC, 256], F16, kind="ExternalInput")
    fc2W = nc.dram_tensor("fc2W", [256, 64], F16, kind="ExternalInput")
    fc2b = nc.dram_tensor("fc2b", [GPC, 64], F16, kind="ExternalInput")
    outW = nc.dram_tensor("outW", [64, 1], F16, kind="ExternalInput")
    outb = nc.dram_tensor("outb", [16, 1], F32, kind="ExternalInput")
    out = nc.dram_tensor("out", [GPC, 1], F32, kind="ExternalOutput")

    with tile.TileContext(nc) as tc:
        with ExitStack() as ctx:
            const = ctx.enter_context(tc.tile_pool(name="const", bufs=1))
            xpool = ctx.enter_context(tc.tile_pool(name="x", bufs=3))
            opool = ctx.enter_context(tc.tile_pool(name="o", bufs=3))
            upool = ctx.enter_context(tc.tile_pool(name="u", bufs=2))
            idxp = ctx.enter_context(tc.tile_pool(name="idx", bufs=3))
            gp = ctx.enter_context(tc.tile_pool(name="g", bufs=3))
            wp = ctx.enter_context(tc.tile_pool(name="wz", bufs=3))
            sp = ctx.enter_context(tc.tile_pool(name="s", bufs=6))
            fin = ctx.enter_context(tc.tile_pool(name="fin", bufs=4))
            ybp = ctx.enter_context(tc.tile_pool(name="yb", bufs=3))
            psum = ctx.enter_context(tc.tile_pool(name="ps", bufs=psum_bufs, space="PSUM"))
            fcps = ctx.enter_context(tc.tile_pool(name="fcps", bufs=1, space="PSUM"))
            ppool = ctx.enter_context(tc.tile_pool(name="ppool", bufs=1, space="PSUM"))

            iota_sb = const.tile([P, P], F16)
            nc.sync.dma_start(iota_sb[:], iota[:, :])
            ident_sb = const.tile([P, P], F16)
            nc.sync.dma_start(ident_sb[:], ident[:, :])
            ones_sb = const.tile([1, P], F16)
            nc.sync.dma_start(ones_sb[:], onescol[:, :])
            brep_sb = const.tile([P, 2, 384], F16)
            for b in range(2):
                nc.sync.dma_start(brep_sb[:, b, :], brep3[b, :, :])
            pbin_sb = const.tile([P, 2, NT, GPC], F16)
            for b in range(2):
                nc.sync.dma_start(pbin_sb[:, b, :, :],
                                  pbin[b].rearrange("t p g -> p t g"))
            pfcW_sb = const.tile([P, 2, P], F16)
            for b in range(2):
                nc.sync.dma_start(pfcW_sb[:, b, :], pfcW[b, :, :])
            pfcb_sb = const.tile([GPC, 2, P], F16)
            for b in range(2):
                nc.sync.dma_start(pfcb_sb[:, b, :], pfcb[b, :, :])
            fc1W_sb = const.tile([P, 2, 256], F16)
            nc.sync.dma_start(fc1W_sb[:, 0, :], fc1W[0:128, :])
            nc.sync.dma_start(fc1W_sb[:, 1, :], fc1W[128:256, :])
            fc1b_sb = const.tile([GPC, 256], F16)
            nc.sync.dma_start(fc1b_sb[:], fc1b[:, :])
            fc2W_sb = const.tile([P, 2, 64], F16)
            nc.sync.dma_start(fc2W_sb[:, 0, :], fc2W[0:128, :])
            nc.sync.dma_start(fc2W_sb[:, 1, :], fc2W[128:256, :])
            fc2b_sb = const.tile([GPC, 64], F16)
            nc.sync.dma_start(fc2b_sb[:], fc2b[:, :])
            outW_sb = const.tile([64, 1], F16)
            nc.sync.dma_start(outW_sb[:], outW[:, :])
            outb_sb = const.tile([GPC, 1], F32)
            nc.sync.dma_start(outb_sb[:], outb[:, :])

            wt = []
            for b in range(2):
                w = const.tile([P, 8, 130], F16, tag=f"w{b}", name=f"w{b}")
                nc.sync.dma_start(w[:, :, :], Wa[b].rearrange("(k p) j -> p k j", p=P))
                wt.append(w)

            uB_all = const.tile([P, 2, NT, P], F16, tag="uB", name="uB_all")

            for rep in range(reps):
              for b in range(2):
                for t in range(NT):
                    xt = xpool.tile([P, 8, P], F16, tag="xt", name=f"xt{rep}_{b}_{t}")
                    nc.sync.dma_start(
                        xt[:, :, :],
                        xT[b][:, t * P:(t + 1) * P].rearrange("(k p) j -> p k j", p=P))
                    ps = psum.tile([P, 130], F32, tag="agg", name=f"p1ps{rep}_{b}_{t}")
                    for k in range(8):
                        nc.tensor.matmul(out=ps[:], lhsT=xt[:, k, :], rhs=wt[b][:, k, :],
                                         start=(k == 0), stop=(k == 7))
                    sb = opool.tile([P, 130], F16, tag="sb", name=f"sb{rep}_{b}_{t}")
                    nc.vector.tensor_copy(sb[:, 0:128], ps[:, 0:128])
                    nc.vector.tensor_copy(sb[:, 129:130], ps[:, 128:129])
                    nc.vector.memset(sb[:, 128:129], 1.0)
                    nc.sync.dma_start(tloc[b][t * P:(t + 1) * P, 0:130], sb[:])
                    uc = opool.tile([P, 1], F16, tag="uc", name=f"uc{rep}_{b}_{t}")
                    nc.scalar.activation(uc[:], ps[:, 129:130], AF.Exp, scale=-0.8)
                    urow_ps = fcps.tile([1, P], F16, tag="fc", name=f"urps{rep}_{b}_{t}")
                    nc.tensor.transpose(out=urow_ps[:], in_=uc[:], identity=ident_sb[:, :])
                    urow = upool.tile([1, P], F16, tag="ur", name=f"ur{rep}_{b}_{t}")
                    nc.vector.tensor_copy(urow[:], urow_ps[:])
                    ub_ps = fcps.tile([P, P], F32, tag="fc", name=f"ubps{rep}_{b}_{t}")
                    nc.tensor.matmul(out=ub_ps[:], lhsT=ones_sb[:], rhs=urow[:],
                                     start=True, stop=True)
                    nc.vector.tensor_copy(uB_all[:, b, t, :], ub_ps[:])

              for b in range(2):
                nc.gpsimd.collective_compute(
                    "AllGather", OP.bypass,
                    replica_groups=[list(range(CORES))],
                    ins=[tloc[b][:, :]], outs=[tbl[b][:, :]],
                )

              poolps = []
              for b in range(2):
                pool_t = ppool.tile([P, GPC], F32, tag=f"pool{b}", name=f"poolps{rep}_{b}")
                poolps.append(pool_t)

              live_ps = {}
              live_yb = {}

              def finalize(sid, rep=rep, poolps=poolps, live_ps=live_ps, live_yb=live_yb):
                br, t, s = slot_list[sid]
                ps = live_ps.pop(sid)
                s_sb = fin.tile([P, 1], F32, tag="ssb", name=f"ssb{rep}_{sid}")
                nc.vector.tensor_scalar(out=s_sb[:], in0=ps[:, 128:129],
                                        scalar1=1e-12, scalar2=None, op0=OP.max)
                r_sb = fin.tile([P, 1], F32, tag="rsb", name=f"rsb{rep}_{sid}")
                nc.vector.reciprocal(r_sb[:], s_sb[:])
                if (br, t) not in live_yb:
                    live_yb[(br, t)] = [ybp.tile([P, 384], F16, tag="ybuf",
                                                 name=f"ybuf{rep}_{br}_{t}"), 0]
                ybe = live_yb[(br, t)]
                yb = ybe[0]
                nc.vector.tensor_scalar(out=yb[:, s * 128:(s + 1) * 128],
                                        in0=ps[:, 0:128],
                                        scalar1=r_sb[:, 0:1], scalar2=None,
                                        op0=OP.mult)
                ybe[1] += 1
                if ybe[1] == 3:
                    live_yb.pop((br, t))
                    yb2 = fin.tile([P, 384], F16, tag="yb2", name=f"yb2_{rep}_{sid}")
                    nc.vector.tensor_tensor(out=yb2[:], in0=yb[:], in1=brep_sb[:, br, :], op=OP.add)
                    t2 = fin.tile([P, 384], F16, tag="t2", name=f"t2_{rep}_{sid}")
                    nc.vector.tensor_scalar(out=t2[:], in0=yb2[:], scalar1=0.01,
                                            scalar2=None, op0=OP.mult)
                    m = fin.tile([P, 384], F16, tag="m", name=f"m_{rep}_{sid}")
                    nc.vector.tensor_tensor(out=m[:], in0=yb2[:], in1=t2[:], op=OP.max)
                    hs = fin.tile([P, 128], F16, tag="hs", name=f"hs_{rep}_{sid}")
                    nc.vector.tensor_tensor(out=hs[:], in0=m[:, 0:128], in1=m[:, 128:256], op=OP.add)
                    nc.vector.tensor_tensor(out=hs[:], in0=hs[:], in1=m[:, 256:384], op=OP.add)
                    nc.tensor.matmul(out=poolps[br][:], lhsT=hs[:],
                                     rhs=pbin_sb[:, br, t, :],
                                     start=(t == emitted_t[br][0]),
                                     stop=(t == emitted_t[br][-1]))

              for k in range(NSUP):
                sm = sup_meta[k]
                br = sm['br']
                nA, nB = sm['nA'], sm['nB']
                ia = idxp.tile([P, 512], I16, tag="ia", name=f"ia{rep}_{k}")
                if nA:
                    nc.sync.dma_start(ia[:, 0:nA * 8], idxA[k, :, 0:nA * 8])
                ib = idxp.tile([P, 512], I16, tag="ib", name=f"ib{rep}_{k}")
                if nB:
                    nc.sync.dma_start(ib[:, 0:nB * 8], idxB[k, :, 0:nB * 8])
                dl = idxp.tile([P, SUPC], F32, tag="dl", name=f"dl{rep}_{k}")
                nc.sync.dma_start(dl[:], dstl[k, :, :])

                g = gp.tile([P, SUPC, 130], F16, tag="g", name=f"g{rep}_{k}")
                if nA:
                    nc.gpsimd.dma_gather(
                        out_ap=g[:, 0:nA, :], in_ap=tbl[br][0:SPLIT, 0:130],
                        idxs_ap=ia[:, 0:nA * 8], num_idxs=nA * P, num_idxs_reg=nA * P,
                        elem_size=130, elem_step=256, single_packet=False)
                if nB:
                    nc.gpsimd.dma_gather(
                        out_ap=g[:, nA:nA + nB, :], in_ap=tbl[br][HIBASE:NPAD, 0:130],
                        idxs_ap=ib[:, 0:nB * 8], num_idxs=nB * P, num_idxs_reg=nB * P,
                        elem_size=130, elem_step=256, single_packet=False)

                A_t = wp.tile([P, SUPC], F32, tag="A", name=f"A{rep}_{k}")
                nc.scalar.activation(A_t[:, 0:nA + nB], g[:, 0:nA + nB, 129], AF.Exp)
                B_t = wp.tile([P, SUPC], F32, tag="B", name=f"B{rep}_{k}")
                nc.scalar.activation(B_t[:, 0:nA + nB], g[:, 0:nA + nB, 129], AF.Exp, scale=0.2)
                negd = wp.tile([P, SUPC], F32, tag="negd", name=f"negd{rep}_{k}")
                nc.vector.tensor_scalar(out=negd[:], in0=dl[:], scalar1=-1.0, scalar2=None, op0=OP.mult)

                for j in range(SUPC):
                    sid, start, stop, use_act = plan[k][j]
                    if sid < 0:
                        continue
                    br_, t_, s_ = slot_list[sid]
                    if start:
                        live_ps[sid] = psum.tile([P, 129], F32, tag="agg", name=f"aggps{rep}_{sid}")
                    ps = live_ps[sid]
                    V = sp.tile([P, P], F16, tag="V", name=f"V{rep}_{k}_{j}")
                    nc.vector.tensor_scalar(out=V[:], in0=uB_all[:, br_, t_, :],
                                            scalar1=B_t[:, j:j + 1],
                                            scalar2=A_t[:, j:j + 1],
                                            op0=OP.mult, op1=OP.max)
                    if use_act:
                        a_t = sp.tile([P, P], F16, tag="a", name=f"a{rep}_{k}_{j}")
                        nc.scalar.activation(a_t[:], iota_sb[:], AF.Abs,
                                             bias=negd[:, j:j + 1])
                        o_t = sp.tile([P, P], F16, tag="oh", name=f"oh{rep}_{k}_{j}")
                        nc.scalar.activation(o_t[:], a_t[:], AF.Relu,
                                             bias=1.0, scale=-1.0)
                    else:
                        o_t = sp.tile([P, P], F16, tag="oh", name=f"oh{rep}_{k}_{j}")
                        nc.vector.tensor_scalar(out=o_t[:], in0=iota_sb[:],
                                                scalar1=dl[:, j:j + 1],
                                                scalar2=None, op0=OP.is_equal)
                    S = sp.tile([P, P], F16, tag="S", name=f"S{rep}_{k}_{j}")
                    nc.vector.tensor_tensor(out=S[:], in0=o_t[:], in1=V[:], op=OP.mult)
                    nc.tensor.matmul(out=ps[:], lhsT=S[:], rhs=g[:, j, 0:129],
                                     start=start, stop=stop)
                    if stop:
                        finalize(sid)

              xT_sb = []
              for b in range(2):
                pT = fin.tile([P, GPC], F16, tag="pT", name=f"pT{rep}_{b}")
                nc.vector.tensor_copy(pT[:], poolps[b][:])
                ps1 = fcps.tile([GPC, P], F32, tag="fc", name=f"ps1_{rep}_{b}")
                nc.tensor.matmul(out=ps1[:], lhsT=pT[:], rhs=pfcW_sb[:, b, :], start=True, stop=True)
                xb = fin.tile([GPC, P], F16, tag="xb", name=f"xb{rep}_{b}")
                nc.vector.tensor_tensor(out=xb[:], in0=ps1[:], in1=pfcb_sb[:, b, :], op=OP.add)
                t2 = fin.tile([GPC, P], F16, tag="xbt", name=f"xbt{rep}_{b}")
                nc.vector.tensor_scalar(out=t2[:], in0=xb[:], scalar1=0.01, scalar2=None, op0=OP.mult)
                nc.vector.tensor_tensor(out=xb[:], in0=xb[:], in1=t2[:], op=OP.max)
                psT = fcps.tile([P, GPC], F16, tag="fc", name=f"psT{rep}_{b}")
                nc.tensor.transpose(out=psT[:], in_=xb[:], identity=ident_sb[0:GPC, 0:GPC])
                xTs = fin.tile([P, GPC], F16, tag=f"xT{b}", name=f"xTs{rep}_{b}")
                nc.vector.tensor_copy(xTs[:], psT[:])
                xT_sb.append(xTs)

              ps2 = fcps.tile([GPC, 256], F32, tag="fc", name=f"ps2_{rep}")
              nc.tensor.matmul(out=ps2[:], lhsT=xT_sb[0][:], rhs=fc1W_sb[:, 0, :], start=True, stop=False)
              nc.tensor.matmul(out=ps2[:], lhsT=xT_sb[1][:], rhs=fc1W_sb[:, 1, :], start=False, stop=True)
              y1 = fin.tile([GPC, 256], F16, tag="y1", name=f"y1_{rep}")
              nc.vector.tensor_tensor(out=y1[:], in0=ps2[:], in1=fc1b_sb[:], op=OP.add)
              t2 = fin.tile([GPC, 256], F16, tag="y1t", name=f"y1t_{rep}")
              nc.vector.tensor_scalar(out=t2[:], in0=y1[:], scalar1=0.01, scalar2=None, op0=OP.mult)
              nc.vector.tensor_tensor(out=y1[:], in0=y1[:], in1=t2[:], op=OP.max)

              y1T = []
              for hlf in range(2):
                psT = fcps.tile([P, GPC], F16, tag="fc", name=f"psTy{rep}_{hlf}")
                nc.tensor.transpose(out=psT[:], in_=y1[:, hlf * 128:(hlf + 1) * 128],
                                    identity=ident_sb[0:GPC, 0:GPC])
                yt = fin.tile([P, GPC], F16, tag=f"y1T{hlf}", name=f"y1T{rep}_{hlf}")
                nc.vector.tensor_copy(yt[:], psT[:])
                y1T.append(yt)

              ps3 = fcps.tile([GPC, 64], F32, tag="fc", name=f"ps3_{rep}")
              nc.tensor.matmul(out=ps3[:], lhsT=y1T[0][:], rhs=fc2W_sb[:, 0, :], start=True, stop=False)
              nc.tensor.matmul(out=ps3[:], lhsT=y1T[1][:], rhs=fc2W_sb[:, 1, :], start=False, stop=True)
              y2 = fin.tile([GPC, 64], F16, tag="y2", name=f"y2_{rep}")
              nc.vector.tensor_tensor(out=y2[:], in0=ps3[:], in1=fc2b_sb[:], op=OP.add)
              t2 = fin.tile([GPC, 64], F16, tag="y2t", name=f"y2t_{rep}")
              nc.vector.tensor_scalar(out=t2[:], in0=y2[:], scalar1=0.01, scalar2=None, op0=OP.mult)
              nc.vector.tensor_tensor(out=y2[:], in0=y2[:], in1=t2[:], op=OP.max)

              psT = fcps.tile([64, GPC], F16, tag="fc", name=f"psTy2_{rep}")
              nc.tensor.transpose(out=psT[:], in_=y2[:], identity=ident_sb[0:GPC, 0:GPC])
              y2T = fin.tile([64, GPC], F16, tag="y2T", name=f"y2T_{rep}")
              nc.vector.tensor_copy(y2T[:], psT[:])

              ps4 = fcps.tile([GPC, 1], F32, tag="fc", name=f"ps4_{rep}")
              nc.tensor.matmul(out=ps4[:], lhsT=y2T[:], rhs=outW_sb[:], start=True, stop=True)
              o_sb = fin.tile([GPC, 1], F32, tag="o", name=f"o_sb{rep}")
              nc.scalar.activation(o_sb[:], ps4[:], AF.Sigmoid, bias=outb_sb[:, 0:1])
              nc.sync.dma_start(out[:, :], o_sb[:])
    nc.compile()
    return nc


def host_inputs(inputs, pp):
    NPC = pp['NPC']
    wa = []
    for b in range(2):
        W = np.asarray(inputs[f'W{b+1}'], np.float64)
        a_s = np.asarray(inputs[f'a{b+1}_src'], np.float64)
        a_d = np.asarray(inputs[f'a{b+1}_dst'], np.float64)
        w_aug = np.concatenate([W, (W @ a_s)[:, None], (W @ a_d)[:, None]], axis=1)
        wa.append(w_aug.astype(np.float16))
    xs = [np.asarray(inputs['pro1_x']), np.asarray(inputs['pro2_x'])]

    iota = np.tile(np.arange(P, dtype=np.float16)[None, :], (P, 1))
    ident = np.eye(P, dtype=np.float16)
    onescol = np.ones((1, P), np.float16)
    brep3 = np.zeros((2, P, 384), np.float16)
    for b in range(2):
        bb = np.asarray(inputs[f'b{b+1}'], np.float32).astype(np.float16)
        brep3[b] = np.tile(bb[None, :], (P, 3))
    pfcW = np.stack([np.asarray(inputs['p1fc_W']), np.asarray(inputs['p2fc_W'])]).astype(np.float16)
    pfcb = np.stack([
        np.tile(np.asarray(inputs['p1fc_b'])[None, :], (GPC, 1)),
        np.tile(np.asarray(inputs['p2fc_b'])[None, :], (GPC, 1)),
    ]).astype(np.float16)
    fc1W = np.asarray(inputs['fc1_W']).astype(np.float16)
    fc1b = np.tile(np.asarray(inputs['fc1_b'])[None, :], (GPC, 1)).astype(np.float16)
    fc2W = np.asarray(inputs['fc2_W']).astype(np.float16)
    fc2b = np.tile(np.asarray(inputs['fc2_b'])[None, :], (GPC, 1)).astype(np.float16)
    outW = np.asarray(inputs['out_W']).astype(np.float16)
    outb = np.tile(np.asarray(inputs['out_b']).reshape(1, 1), (GPC, 1)).astype(np.float32)

    maps = []
    for c in range(CORES):
        m = dict(
            idxA=pp['idxA'][c], idxB=pp['idxB'][c],
            dstl=pp['dstl'][c], pbin=pp['pbin'][c],
            iota=iota, ident=ident, onescol=onescol, brep3=brep3,
            pfcW=pfcW, pfcb=pfcb, fc1W=fc1W, fc1b=fc1b,
            fc2W=fc2W, fc2b=fc2b, outW=outW, outb=outb,
        )
        for b in range(2):
            lo, hi = pp['core_lo'][b][c], pp['core_hi'][b][c]
            xt = np.zeros((1024, NPC), np.float16)
            xt[:, 0:hi - lo] = xs[b][lo:hi].T.astype(np.float16)
            m[f'xT{b+1}'] = xt
            m[f'W{b+1}aug'] = wa[b]
        maps.append(m)
    return maps


_CACHE = {}


def _cache_key(pp):
    return (pp['NT'], pp['NSUP'],
            tuple(int(x) for x in pp['CHh'].reshape(-1)))


def get_nc(pp, **kw):
    key = (_cache_key(pp), tuple(sorted(kw.items())))
    if key not in _CACHE:
        _CACHE[key] = build_fused(pp, **kw)
    return _CACHE[key]


def kernel(**inputs):
    from concourse import bass_utils
    inputs = {k: np.asarray(v) for k, v in inputs.items()}
    pp = prep(inputs)
    nc = get_nc(pp)
    maps = host_inputs(inputs, pp)
    res = bass_utils.run_bass_kernel_spmd(nc, maps, core_ids=list(range(CORES)))
    return np.concatenate([res.results[c]['out'] for c in range(CORES)], axis=0)


# revision 4
# speedup vs baseline: 3.2214x; 1.7605x over previous
"""MultiHopAttGNN on 8 Trainium2 NeuronCores — fused NEFF, v4.

v4 over v3 (kernel_fused):
  - Per-half chunk counts decoupled: chunks(slot, half) = max over cores of
    ceil(n_edges/128) per half independently (was max over both halves),
    and superchunks take a variable A/B split -> ~10% fewer gather
    descriptors (the gather is HBM-latency-bound at ~283 ns/desc/engine).
  - Edges within each (slot, half) sorted by src index for HBM row
    locality; pad lanes duplicate the previous real src row (row hit).
  - Partial last gathers use exact compile-time num_idxs.
See kernel_fused.py docstring for the pipeline and the ad-gather
elimination math.
"""
import sys
sys.path.insert(0, '/opt/trn_rl_repo')
import inspect
import textwrap
from contextlib import ExitStack

import numpy as np

import concourse.bass as bass
import concourse.bacc as bacc
import concourse.mybir as mybir
import concourse.tile as tile

_src = textwrap.dedent(inspect.getsource(bass.BassGpSimd.dma_gather))
_src = _src.replace("elem_size_bytes > 0 and elem_size_bytes % 256 == 0",
                    "elem_size_bytes > 0")
_ns = vars(bass).copy()
exec(compile(_src, "<dma_gather_patched>", "exec"), _ns)
bass.BassGpSimd.dma_gather = _ns["dma_gather"]

CORES = 8
P = 128
SUPC = 64            # chunks per superchunk
D = 128
NG = 128
GPC = NG // CORES
F16 = mybir.dt.float16
F32 = mybir.dt.float32
I16 = mybir.dt.int16
AF = mybir.ActivationFunctionType
OP = mybir.AluOpType


def wrap_idx(idx_flat):
    n = idx_flat.shape[0]
    base = idx_flat.reshape(n // 16, 16).T.astype(np.int16)
    return np.tile(base, (8, 1))


def prep(inputs):
    out = {}
    batches = [np.asarray(inputs['pro1_batch']), np.asarray(inputs['pro2_batch'])]
    N = batches[0].shape[0]

    bounds = [np.searchsorted(b, np.arange(NG + 1)) for b in batches]
    core_lo = [[int(bounds[br][c * GPC]) for c in range(CORES)] for br in range(2)]
    core_hi = [[int(bounds[br][(c + 1) * GPC]) for c in range(CORES)] for br in range(2)]
    nodes_c = [[core_hi[br][c] - core_lo[br][c] for c in range(CORES)] for br in range(2)]
    NT = max(max((n + P - 1) // P for n in nodes_c[br]) for br in range(2))
    NPC = NT * P
    NPAD = CORES * NPC
    HIBASE = max(0, NPAD - 32000)
    SPLIT = max(HIBASE, (NPAD // 2 // 128) * 128) if NPAD < 32000 else 25600
    assert SPLIT < 32768 and NPAD - HIBASE <= 32000 and HIBASE <= SPLIT
    out.update(NT=NT, NPC=NPC, NPAD=NPAD, SPLIT=SPLIT, HIBASE=HIBASE,
               core_lo=core_lo, core_hi=core_hi)

    gid = []
    for br in range(2):
        g = np.zeros(N, np.int64)
        for c in range(CORES):
            lo, hi = core_lo[br][c], core_hi[br][c]
            g[lo:hi] = c * NPC + (np.arange(lo, hi) - lo)
        gid.append(g)

    # per-(branch, tile, set, half) edge lists per core, sorted by src
    CHh = np.zeros((2, NT, 3, 2), np.int64)
    ed = {}   # (br,t,s,h,c) -> (src_idx_for_half, dstl_local)
    for br in range(2):
        for s in range(3):
            ei = np.asarray(inputs[f'pro{br+1}_ei{s+1}'])
            loops = np.arange(N, dtype=ei.dtype)
            src = np.concatenate([ei[0], loops])
            dst = np.concatenate([ei[1], loops])
            sg = gid[br][src]
            dg = gid[br][dst]
            core_of = dg // NPC
            for c in range(CORES):
                m = core_of == c
                sc, dc = sg[m], dg[m]
                dl = dc - c * NPC
                t_of = dl // P
                ha = sc < SPLIT
                for h in range(2):
                    mh = ha if h == 0 else ~ha
                    sch = sc[mh] if h == 0 else sc[mh] - HIBASE
                    dlh = dl[mh]
                    th = t_of[mh]
                    for t in range(NT):
                        mt = th == t
                        scm, dlm = sch[mt], dlh[mt]
                        order = np.argsort(scm, kind='stable')
                        scm, dlm = scm[order], dlm[order]
                        ed[(br, t, s, h, c)] = (scm, dlm - t * P)
                        CHh[br, t, s, h] = max(CHh[br, t, s, h],
                                               (scm.shape[0] + P - 1) // P)
    out['CHh'] = CHh

    # chunk lists per branch (A-half list and B-half list, in (t,s) order)
    slot_list = []          # sid -> (br, t, s)
    sid_of = {}
    for br in range(2):
        for t in range(NT):
            for s in range(3):
                sid_of[(br, t, s)] = len(slot_list)
                slot_list.append((br, t, s))
    out['slot_list'] = slot_list

    half_lists = {0: ([], []), 1: ([], [])}   # br -> (Alist, Blist)
    for br in range(2):
        for t in range(NT):
            for s in range(3):
                for h in range(2):
                    lst = half_lists[br][h]
                    for cseq in range(int(CHh[br, t, s, h])):
                        lst.append((sid_of[(br, t, s)], h, cseq))

    # assign chunks to superchunks as groups of whole slots (a slot's A and B
    # chunks land in one superchunk -> few live PSUM accumulators). Partial
    # superchunks cost nothing: gathers use exact chunk counts.
    sup_meta = []    # per superchunk: dict(br, nA, nB, chunks=[(sid,h,cseq)...])
    for br in range(2):
        # per-slot (A-chunks, B-chunks) in sid order
        slots_br = {}
        order = []
        for h in range(2):
            for item in half_lists[br][h]:
                sid = item[0]
                if sid not in slots_br:
                    slots_br[sid] = ([], [])
                    order.append(sid)
                slots_br[sid][h].append(item)
        order.sort()
        group_a, group_b = [], []
        def emit(group_a=None, group_b=None, _sup=sup_meta, _br=br):
            pass
        i = 0
        while i < len(order):
            sid = order[i]
            ach, bch = slots_br[sid]
            n_new = len(ach) + len(bch)
            cur = len(group_a) + len(group_b)
            if cur > 0 and cur + n_new > SUPC:
                chunks = group_a + group_b
                nA, nB = len(group_a), len(group_b)
                chunks += [None] * (SUPC - len(chunks))
                sup_meta.append(dict(br=br, nA=nA, nB=nB, chunks=chunks))
                group_a, group_b = [], []
                continue
            if n_new > SUPC:
                # pathological giant slot: split across superchunks
                rem_a, rem_b = list(ach), list(bch)
                while rem_a or rem_b:
                    room = SUPC - len(group_a) - len(group_b)
                    ta = rem_a[:room]
                    rem_a = rem_a[len(ta):]
                    room -= len(ta)
                    tb = rem_b[:room]
                    rem_b = rem_b[len(tb):]
                    group_a.extend(ta)
                    group_b.extend(tb)
                    if len(group_a) + len(group_b) == SUPC:
                        chunks = group_a + group_b
                        sup_meta.append(dict(br=br, nA=len(group_a), nB=len(group_b), chunks=chunks))
                        group_a, group_b = [], []
                i += 1
                continue
            group_a.extend(ach)
            group_b.extend(bch)
            i += 1
        if group_a or group_b:
            chunks = group_a + group_b
            nA, nB = len(group_a), len(group_b)
            chunks += [None] * (SUPC - len(chunks))
            sup_meta.append(dict(br=br, nA=nA, nB=nB, chunks=chunks))
    NSUP = len(sup_meta)
    out.update(NSUP=NSUP, sup_meta=sup_meta)

    # start/stop per chunk in emission order; max live psum
    nslots = len(slot_list)
    first_pos = [None] * nslots
    last_pos = [None] * nslots
    pos = 0
    pos_of = {}
    for k, sm in enumerate(sup_meta):
        for j in range(SUPC):
            ch = sm['chunks'][j]
            if ch is None:
                pos += 1
                continue
            sid = ch[0]
            if first_pos[sid] is None:
                first_pos[sid] = pos
            last_pos[sid] = pos
            pos_of[(k, j)] = pos
            pos += 1
    live = 0
    max_live = 0
    pos = 0
    for k, sm in enumerate(sup_meta):
        for j in range(SUPC):
            ch = sm['chunks'][j]
            if ch is not None:
                sid = ch[0]
                if pos == first_pos[sid]:
                    live += 1
                    max_live = max(max_live, live)
                if pos == last_pos[sid]:
                    live -= 1
            pos += 1
    out['max_live'] = max_live

    plan = []
    pos = 0
    cnt = 0
    act_frac = 0.55
    for k, sm in enumerate(sup_meta):
        sup = []
        for j in range(SUPC):
            ch = sm['chunks'][j]
            if ch is None:
                sup.append((-1, False, False, False))
                pos += 1
                continue
            sid = ch[0]
            start = pos == first_pos[sid]
            stop = pos == last_pos[sid]
            use_act = (cnt % 100) < int(act_frac * 100)
            cnt += 1
            sup.append((sid, start, stop, use_act))
            pos += 1
        plan.append(sup)
    out['plan'] = plan

    # per-core idx / dstl tables
    ship_idxA = np.zeros((CORES, NSUP, P, 512), np.int16)
    ship_idxB = np.zeros((CORES, NSUP, P, 512), np.int16)
    ship_dstl = np.full((CORES, NSUP, P, SUPC), 999.0, np.float32)
    for c in range(CORES):
        # pad each (sid,h) to CHh*128 with dup-last idx and dstl=999
        padded = {}
        for (br, t, s, h, cc), (scm, dlm) in ed.items():
            if cc != c:
                continue
            nch = int(CHh[br, t, s, h])
            idx_pad = np.zeros(nch * P, np.int64)
            dl_pad = np.full(nch * P, 999.0, np.float32)
            n = scm.shape[0]
            idx_pad[:n] = scm
            if n > 0 and n < nch * P:
                idx_pad[n:] = scm[-1]
            dl_pad[:n] = dlm
            padded[(sid_of[(br, t, s)], h)] = (idx_pad, dl_pad)
        for k, sm in enumerate(sup_meta):
            nA, nB = sm['nA'], sm['nB']
            a_idx = np.zeros(SUPC * P // 2 * 2, np.int64)  # scratch
            aflat = []
            bflat = []
            for j in range(SUPC):
                ch = sm['chunks'][j]
                if ch is None:
                    continue
                sid, h, cseq = ch
                idx_pad, dl_pad = padded[(sid, h)]
                seg_i = idx_pad[cseq * P:(cseq + 1) * P]
                seg_d = dl_pad[cseq * P:(cseq + 1) * P]
                (aflat if h == 0 else bflat).append(seg_i)
                ship_dstl[c, k, :, j] = seg_d
            if aflat:
                fa = np.concatenate(aflat)
                ship_idxA[c, k, :, 0:fa.shape[0] // 16] = wrap_idx(fa)[:, 0:fa.shape[0] // 16]
            if bflat:
                fb = np.concatenate(bflat)
                ship_idxB[c, k, :, 0:fb.shape[0] // 16] = wrap_idx(fb)[:, 0:fb.shape[0] // 16]
    out.update(idxA=ship_idxA, idxB=ship_idxB, dstl=ship_dstl)

    pbin = np.zeros((CORES, 2, NT, P, GPC), np.float16)
    for br in range(2):
        cnts = np.bincount(batches[br], minlength=NG).astype(np.float64)
        inv = 1.0 / np.maximum(cnts, 1.0)
        for c in range(CORES):
            lo, hi = core_lo[br][c], core_hi[br][c]
            loc_graph = batches[br][lo:hi] - c * GPC
            loc_node = np.arange(hi - lo)
            pbin[c, br, loc_node // P, loc_node % P, loc_graph] = \
                inv[batches[br][lo:hi]].astype(np.float16)
    out['pbin'] = pbin
    return out


def build_fused(pp, reps=1, shared_tbl=True, psum_bufs=None):
    NT, NPC, NPAD, NSUP = pp['NT'], pp['NPC'], pp['NPAD'], pp['NSUP']
    SPLIT, HIBASE = pp['SPLIT'], pp['HIBASE']
    plan = pp['plan']
    slot_list = pp['slot_list']
    sup_meta = pp['sup_meta']
    CHh = pp['CHh']
    emitted_t = {b: [t for t in range(NT) if CHh[b, t].sum() > 0] for b in range(2)}
    if psum_bufs is None:
        psum_bufs = 5
    assert pp['max_live'] <= psum_bufs

    nc = bacc.Bacc("TRN2", target_bir_lowering=False, debug=False, num_devices=CORES)
    xT = [nc.dram_tensor(f"xT{b+1}", [1024, NPC], F16, kind="ExternalInput") for b in range(2)]
    Wa = [nc.dram_tensor(f"W{b+1}aug", [1024, 130], F16, kind="ExternalInput") for b in range(2)]
    tloc = [nc.dram_tensor(f"tloc{b+1}", [NPC, 256], F16) for b in range(2)]
    tkw = dict(addr_space="Shared") if shared_tbl else {}
    tbl = [nc.dram_tensor(f"tfull{b+1}", [NPAD, 256], F16, **tkw) for b in range(2)]
    idxA = nc.dram_tensor("idxA", [NSUP, P, 512], I16, kind="ExternalInput")
    idxB = nc.dram_tensor("idxB", [NSUP, P, 512], I16, kind="ExternalInput")
    dstl = nc.dram_tensor("dstl", [NSUP, P, SUPC], F32, kind="ExternalInput")
    pbin = nc.dram_tensor("pbin", [2, NT, P, GPC], F16, kind="ExternalInput")
    iota = nc.dram_tensor("iota", [P, P], F16, kind="ExternalInput")
    ident = nc.dram_tensor("ident", [P, P], F16, kind="ExternalInput")
    onescol = nc.dram_tensor("onescol", [1, P], F16, kind="ExternalInput")
    brep3 = nc.dram_tensor("brep3", [2, P, 384], F16, kind="ExternalInput")
    pfcW = nc.dram_tensor("pfcW", [2, P, P], F16, kind="ExternalInput")
    pfcb = nc.dram_tensor("pfcb", [2, GPC, P], F16, kind="ExternalInput")
    fc1W = nc.dram_tensor("fc1W", [256, 256], F16, kind="ExternalInput")
    fc1b = nc.dram_tensor("fc1b", [GPC, 256], F16, kind="ExternalInput")
    fc2W = nc.dram_tensor("fc2W", [256, 64], F16, kind="ExternalInput")
    fc2b = nc.dram_tensor("fc2b", [GPC, 64], F16, kind="ExternalInput")
    outW = nc.dram_tensor("outW", [64, 1], F16, kind="ExternalInput")
    outb = nc.dram_tensor("outb", [16, 1], F32, kind="ExternalInput")
    out = nc.dram_tensor("out", [GPC, 1], F32, kind="ExternalOutput")

    with tile.TileContext(nc) as tc:
        with ExitStack() as ctx:
            const = ctx.enter_context(tc.tile_pool(name="const", bufs=1))
            xpool = ctx.enter_context(tc.tile_pool(name="x", bufs=3))
            opool = ctx.enter_context(tc.tile_pool(name="o", bufs=3))
            upool = ctx.enter_context(tc.tile_pool(name="u", bufs=2))
            idxp = ctx.enter_context(tc.tile_pool(name="idx", bufs=4))
            gp = ctx.enter_context(tc.tile_pool(name="g", bufs=4))
            wp = ctx.enter_context(tc.tile_pool(name="wz", bufs=4))
            sp = ctx.enter_context(tc.tile_pool(name="s", bufs=10))
            fin = ctx.enter_context(tc.tile_pool(name="fin", bufs=4))
            ybp = ctx.enter_context(tc.tile_pool(name="yb", bufs=3))
            psum = ctx.enter_context(tc.tile_pool(name="ps", bufs=psum_bufs, space="PSUM"))
            fcps = ctx.enter_context(tc.tile_pool(name="fcps", bufs=1, space="PSUM"))
            ppool = ctx.enter_context(tc.tile_pool(name="ppool", bufs=1, space="PSUM"))

            iota_sb = const.tile([P, P], F16)
            nc.sync.dma_start(iota_sb[:], iota[:, :])
            ident_sb = const.tile([P, P], F16)
            nc.sync.dma_start(ident_sb[:], ident[:, :])
            ones_sb = const.tile([1, P], F16)
            nc.sync.dma_start(ones_sb[:], onescol[:, :])
            brep_sb = const.tile([P, 2, 384], F16)
            for b in range(2):
                nc.sync.dma_start(brep_sb[:, b, :], brep3[b, :, :])
            pbin_sb = const.tile([P, 2, NT, GPC], F16)
            for b in range(2):
                nc.sync.dma_start(pbin_sb[:, b, :, :],
                                  pbin[b].rearrange("t p g -> p t g"))
            pfcW_sb = const.tile([P, 2, P], F16)
            for b in range(2):
                nc.sync.dma_start(pfcW_sb[:, b, :], pfcW[b, :, :])
            pfcb_sb = const.tile([GPC, 2, P], F16)
            for b in range(2):
                nc.sync.dma_start(pfcb_sb[:, b, :], pfcb[b, :, :])
            fc1W_sb = const.tile([P, 2, 256], F16)
            nc.sync.dma_start(fc1W_sb[:, 0, :], fc1W[0:128, :])
            nc.sync.dma_start(fc1W_sb[:, 1, :], fc1W[128:256, :])
            fc1b_sb = const.tile([GPC, 256], F16)
            nc.sync.dma_start(fc1b_sb[:], fc1b[:, :])
            fc2W_sb = const.tile([P, 2, 64], F16)
            nc.sync.dma_start(fc2W_sb[:, 0, :], fc2W[0:128, :])
            nc.sync.dma_start(fc2W_sb[:, 1, :], fc2W[128:256, :])
            fc2b_sb = const.tile([GPC, 64], F16)
            nc.sync.dma_start(fc2b_sb[:], fc2b[:, :])
            outW_sb = const.tile([64, 1], F16)
            nc.sync.dma_start(outW_sb[:], outW[:, :])
            outb_sb = const.tile([GPC, 1], F32)
            nc.sync.dma_start(outb_sb[:], outb[:, :])

            wt = []
            for b in range(2):
                w = const.tile([P, 8, 130], F16, tag=f"w{b}", name=f"w{b}")
                nc.sync.dma_start(w[:, :, :], Wa[b].rearrange("(k p) j -> p k j", p=P))
                wt.append(w)

            uB_all = const.tile([P, 2, NT, P], F16, tag="uB", name="uB_all")

            for rep in range(reps):
              for b in range(2):
                for t in range(NT):
                    xt = xpool.tile([P, 8, P], F16, tag="xt", name=f"xt{rep}_{b}_{t}")
                    nc.sync.dma_start(
                        xt[:, :, :],
                        xT[b][:, t * P:(t + 1) * P].rearrange("(k p) j -> p k j", p=P))
                    ps = psum.tile([P, 130], F32, tag="agg", name=f"p1ps{rep}_{b}_{t}")
                    for k in range(8):
                        nc.tensor.matmul(out=ps[:], lhsT=xt[:, k, :], rhs=wt[b][:, k, :],
                                         start=(k == 0), stop=(k == 7))
                    sb = opool.tile([P, 130], F16, tag="sb", name=f"sb{rep}_{b}_{t}")
                    nc.vector.tensor_copy(sb[:, 0:128], ps[:, 0:128])
                    nc.vector.tensor_copy(sb[:, 129:130], ps[:, 128:129])
                    nc.vector.memset(sb[:, 128:129], 1.0)
                    nc.sync.dma_start(tloc[b][t * P:(t + 1) * P, 0:130], sb[:])
                    uc = opool.tile([P, 1], F16, tag="uc", name=f"uc{rep}_{b}_{t}")
                    nc.scalar.activation(uc[:], ps[:, 129:130], AF.Exp, scale=-0.8)
                    urow_ps = fcps.tile([1, P], F16, tag="fc", name=f"urps{rep}_{b}_{t}")
                    nc.tensor.transpose(out=urow_ps[:], in_=uc[:], identity=ident_sb[:, :])
                    urow = upool.tile([1, P], F16, tag="ur", name=f"ur{rep}_{b}_{t}")
                    nc.vector.tensor_copy(urow[:], urow_ps[:])
                    ub_ps = fcps.tile([P, P], F32, tag="fc", name=f"ubps{rep}_{b}_{t}")
                    nc.tensor.matmul(out=ub_ps[:], lhsT=ones_sb[:], rhs=urow[:],
                                     start=True, stop=True)
                    nc.vector.tensor_copy(uB_all[:, b, t, :], ub_ps[:])

              for b in range(2):
                nc.gpsimd.collective_compute(
                    "AllGather", OP.bypass,
                    replica_groups=[list(range(CORES))],
                    ins=[tloc[b][:, :]], outs=[tbl[b][:, :]],
                )

              poolps = []
              for b in range(2):
                pool_t = ppool.tile([P, GPC], F32, tag=f"pool{b}", name=f"poolps{rep}_{b}")
                poolps.append(pool_t)

              live_ps = {}
              live_yb = {}

              def finalize(sid, rep=rep, poolps=poolps, live_ps=live_ps, live_yb=live_yb):
                br, t, s = slot_list[sid]
                ps = live_ps.pop(sid)
                s_sb = fin.tile([P, 1], F32, tag="ssb", name=f"ssb{rep}_{sid}")
                nc.vector.tensor_scalar(out=s_sb[:], in0=ps[:, 128:129],
                                        scalar1=1e-12, scalar2=None, op0=OP.max)
                r_sb = fin.tile([P, 1], F32, tag="rsb", name=f"rsb{rep}_{sid}")
                nc.vector.reciprocal(r_sb[:], s_sb[:])
                if (br, t) not in live_yb:
                    live_yb[(br, t)] = [ybp.tile([P, 384], F16, tag="ybuf",
                                                 name=f"ybuf{rep}_{br}_{t}"), 0]
                ybe = live_yb[(br, t)]
                yb = ybe[0]
                nc.vector.tensor_scalar(out=yb[:, s * 128:(s + 1) * 128],
                                        in0=ps[:, 0:128],
                                        scalar1=r_sb[:, 0:1], scalar2=None,
                                        op0=OP.mult)
                ybe[1] += 1
                if ybe[1] == 3:
                    live_yb.pop((br, t))
                    yb2 = fin.tile([P, 384], F16, tag="yb2", name=f"yb2_{rep}_{sid}")
                    nc.vector.tensor_tensor(out=yb2[:], in0=yb[:], in1=brep_sb[:, br, :], op=OP.add)
                    t2 = fin.tile([P, 384], F16, tag="t2", name=f"t2_{rep}_{sid}")
                    nc.vector.tensor_scalar(out=t2[:], in0=yb2[:], scalar1=0.01,
                                            scalar2=None, op0=OP.mult)
                    m = fin.tile([P, 384], F16, tag="m", name=f"m_{rep}_{sid}")
                    nc.vector.tensor_tensor(out=m[:], in0=yb2[:], in1=t2[:], op=OP.max)
                    hs = fin.tile([P, 128], F16, tag="hs", name=f"hs_{rep}_{sid}")
                    nc.vector.tensor_tensor(out=hs[:], in0=m[:, 0:128], in1=m[:, 128:256], op=OP.add)
                    nc.vector.tensor_tensor(out=hs[:], in0=hs[:], in1=m[:, 256:384], op=OP.add)
                    nc.tensor.matmul(out=poolps[br][:], lhsT=hs[:],
                                     rhs=pbin_sb[:, br, t, :],
                                     start=(t == emitted_t[br][0]),
                                     stop=(t == emitted_t[br][-1]))

              for k in range(NSUP):
                sm = sup_meta[k]
                br = sm['br']
                nA, nB = sm['nA'], sm['nB']
                ia = idxp.tile([P, 512], I16, tag="ia", name=f"ia{rep}_{k}")
                if nA:
                    nc.sync.dma_start(ia[:, 0:nA * 8], idxA[k, :, 0:nA * 8])
                ib = idxp.tile([P, 512], I16, tag="ib", name=f"ib{rep}_{k}")
                if nB:
                    nc.sync.dma_start(ib[:, 0:nB * 8], idxB[k, :, 0:nB * 8])
                dl = idxp.tile([P, SUPC], F32, tag="dl", name=f"dl{rep}_{k}")
                nc.sync.dma_start(dl[:], dstl[k, :, :])

                g = gp.tile([P, SUPC, 130], F16, tag="g", name=f"g{rep}_{k}")
                if nA:
                    nc.gpsimd.dma_gather(
                        out_ap=g[:, 0:nA, :], in_ap=tbl[br][0:SPLIT, 0:130],
                        idxs_ap=ia[:, 0:nA * 8], num_idxs=nA * P, num_idxs_reg=nA * P,
                        elem_size=130, elem_step=256, single_packet=False)
                if nB:
                    nc.gpsimd.dma_gather(
                        out_ap=g[:, nA:nA + nB, :], in_ap=tbl[br][HIBASE:NPAD, 0:130],
                        idxs_ap=ib[:, 0:nB * 8], num_idxs=nB * P, num_idxs_reg=nB * P,
                        elem_size=130, elem_step=256, single_packet=False)

                A_t = wp.tile([P, SUPC], F32, tag="A", name=f"A{rep}_{k}")
                nc.scalar.activation(A_t[:, 0:nA + nB], g[:, 0:nA + nB, 129], AF.Exp)
                B_t = wp.tile([P, SUPC], F32, tag="B", name=f"B{rep}_{k}")
                nc.scalar.activation(B_t[:, 0:nA + nB], g[:, 0:nA + nB, 129], AF.Exp, scale=0.2)
                negd = wp.tile([P, SUPC], F32, tag="negd", name=f"negd{rep}_{k}")
                nc.vector.tensor_scalar(out=negd[:], in0=dl[:], scalar1=-1.0, scalar2=None, op0=OP.mult)

                for j in range(SUPC):
                    sid, start, stop, use_act = plan[k][j]
                    if sid < 0:
                        continue
                    br_, t_, s_ = slot_list[sid]
                    if start:
                        live_ps[sid] = psum.tile([P, 129], F32, tag="agg", name=f"aggps{rep}_{sid}")
                    ps = live_ps[sid]
                    V = sp.tile([P, P], F16, tag="V", name=f"V{rep}_{k}_{j}")
                    nc.vector.tensor_scalar(out=V[:], in0=uB_all[:, br_, t_, :],
                                            scalar1=B_t[:, j:j + 1],
                                            scalar2=A_t[:, j:j + 1],
                                            op0=OP.mult, op1=OP.max)
                    if use_act:
                        a_t = sp.tile([P, P], F16, tag="a", name=f"a{rep}_{k}_{j}")
                        nc.scalar.activation(a_t[:], iota_sb[:], AF.Abs,
                                             bias=negd[:, j:j + 1])
                        o_t = sp.tile([P, P], F16, tag="oh", name=f"oh{rep}_{k}_{j}")
                        nc.scalar.activation(o_t[:], a_t[:], AF.Relu,
                                             bias=1.0, scale=-1.0)
                    else:
                        o_t = sp.tile([P, P], F16, tag="oh", name=f"oh{rep}_{k}_{j}")
                        nc.vector.tensor_scalar(out=o_t[:], in0=iota_sb[:],
                                                scalar1=dl[:, j:j + 1],
                                                scalar2=None, op0=OP.is_equal)
                    S = sp.tile([P, P], F16, tag="S", name=f"S{rep}_{k}_{j}")
                    nc.vector.tensor_tensor(out=S[:], in0=o_t[:], in1=V[:], op=OP.mult)
                    nc.tensor.matmul(out=ps[:], lhsT=S[:], rhs=g[:, j, 0:129],
                                     start=start, stop=stop)
                    if stop:
                        finalize(sid)

              xT_sb = []
              for b in range(2):
                pT = fin.tile([P, GPC], F16, tag="pT", name=f"pT{rep}_{b}")
                nc.vector.tensor_copy(pT[:], poolps[b][:])
                ps1 = fcps.tile([GPC, P], F32, tag="fc", name=f"ps1_{rep}_{b}")
                nc.tensor.matmul(out=ps1[:], lhsT=pT[:], rhs=pfcW_sb[:, b, :], start=True, stop=True)
                xb = fin.tile([GPC, P], F16, tag="xb", name=f"xb{rep}_{b}")
                nc.vector.tensor_tensor(out=xb[:], in0=ps1[:], in1=pfcb_sb[:, b, :], op=OP.add)
                t2 = fin.tile([GPC, P], F16, tag="xbt", name=f"xbt{rep}_{b}")
                nc.vector.tensor_scalar(out=t2[:], in0=xb[:], scalar1=0.01, scalar2=None, op0=OP.mult)
                nc.vector.tensor_tensor(out=xb[:], in0=xb[:], in1=t2[:], op=OP.max)
                psT = fcps.tile([P, GPC], F16, tag="fc", name=f"psT{rep}_{b}")
                nc.tensor.transpose(out=psT[:], in_=xb[:], identity=ident_sb[0:GPC, 0:GPC])
                xTs = fin.tile([P, GPC], F16, tag=f"xT{b}", name=f"xTs{rep}_{b}")
                nc.vector.tensor_copy(xTs[:], psT[:])
                xT_sb.append(xTs)

              ps2 = fcps.tile([GPC, 256], F32, tag="fc", name=f"ps2_{rep}")
              nc.tensor.matmul(out=ps2[:], lhsT=xT_sb[0][:], rhs=fc1W_sb[:, 0, :], start=True, stop=False)
              nc.tensor.matmul(out=ps2[:], lhsT=xT_sb[1][:], rhs=fc1W_sb[:, 1, :], start=False, stop=True)
              y1 = fin.tile([GPC, 256], F16, tag="y1", name=f"y1_{rep}")
              nc.vector.tensor_tensor(out=y1[:], in0=ps2[:], in1=fc1b_sb[:], op=OP.add)
              t2 = fin.tile([GPC, 256], F16, tag="y1t", name=f"y1t_{rep}")
              nc.vector.tensor_scalar(out=t2[:], in0=y1[:], scalar1=0.01, scalar2=None, op0=OP.mult)
              nc.vector.tensor_tensor(out=y1[:], in0=y1[:], in1=t2[:], op=OP.max)

              y1T = []
              for hlf in range(2):
                psT = fcps.tile([P, GPC], F16, tag="fc", name=f"psTy{rep}_{hlf}")
                nc.tensor.transpose(out=psT[:], in_=y1[:, hlf * 128:(hlf + 1) * 128],
                                    identity=ident_sb[0:GPC, 0:GPC])
                yt = fin.tile([P, GPC], F16, tag=f"y1T{hlf}", name=f"y1T{rep}_{hlf}")
                nc.vector.tensor_copy(yt[:], psT[:])
                y1T.append(yt)

              ps3 = fcps.tile([GPC, 64], F32, tag="fc", name=f"ps3_{rep}")
              nc.tensor.matmul(out=ps3[:], lhsT=y1T[0][:], rhs=fc2W_sb[:, 0, :], start=True, stop=False)
              nc.tensor.matmul(out=ps3[:], lhsT=y1T[1][:], rhs=fc2W_sb[:, 1, :], start=False, stop=True)
              y2 = fin.tile([GPC, 64], F16, tag="y2", name=f"y2_{rep}")
              nc.vector.tensor_tensor(out=y2[:], in0=ps3[:], in1=fc2b_sb[:], op=OP.add)
              t2 = fin.tile([GPC, 64], F16, tag="y2t", name=f"y2t_{rep}")
              nc.vector.tensor_scalar(out=t2[:], in0=y2[:], scalar1=0.01, scalar2=None, op0=OP.mult)
              nc.vector.tensor_tensor(out=y2[:], in0=y2[:], in1=t2[:], op=OP.max)

              psT = fcps.tile([64, GPC], F16, tag="fc", name=f"psTy2_{rep}")
              nc.tensor.transpose(out=psT[:], in_=y2[:], identity=ident_sb[0:GPC, 0:GPC])
              y2T = fin.tile([64, GPC], F16, tag="y2T", name=f"y2T_{rep}")
              nc.vector.tensor_copy(y2T[:], psT[:])

              ps4 = fcps.tile([GPC, 1], F32, tag="fc", name=f"ps4_{rep}")
              nc.tensor.matmul(out=ps4[:], lhsT=y2T[:], rhs=outW_sb[:], start=True, stop=True)
              o_sb = fin.tile([GPC, 1], F32, tag="o", name=f"o_sb{rep}")
              nc.scalar.activation(o_sb[:], ps4[:], AF.Sigmoid, bias=outb_sb[:, 0:1])
              nc.sync.dma_start(out[:, :], o_sb[:])
    nc.compile()
    return nc


def host_inputs(inputs, pp):
    NPC = pp['NPC']
    wa = []
    for b in range(2):
        W = np.asarray(inputs[f'W{b+1}'], np.float64)
        a_s = np.asarray(inputs[f'a{b+1}_src'], np.float64)
        a_d = np.asarray(inputs[f'a{b+1}_dst'], np.float64)
        w_aug = np.concatenate([W, (W @ a_s)[:, None], (W @ a_d)[:, None]], axis=1)
        wa.append(w_aug.astype(np.float16))
    xs = [np.asarray(inputs['pro1_x']), np.asarray(inputs['pro2_x'])]

    iota = np.tile(np.arange(P, dtype=np.float16)[None, :], (P, 1))
    ident = np.eye(P, dtype=np.float16)
    onescol = np.ones((1, P), np.float16)
    brep3 = np.zeros((2, P, 384), np.float16)
    for b in range(2):
        bb = np.asarray(inputs[f'b{b+1}'], np.float32).astype(np.float16)
        brep3[b] = np.tile(bb[None, :], (P, 3))
    pfcW = np.stack([np.asarray(inputs['p1fc_W']), np.asarray(inputs['p2fc_W'])]).astype(np.float16)
    pfcb = np.stack([
        np.tile(np.asarray(inputs['p1fc_b'])[None, :], (GPC, 1)),
        np.tile(np.asarray(inputs['p2fc_b'])[None, :], (GPC, 1)),
    ]).astype(np.float16)
    fc1W = np.asarray(inputs['fc1_W']).astype(np.float16)
    fc1b = np.tile(np.asarray(inputs['fc1_b'])[None, :], (GPC, 1)).astype(np.float16)
    fc2W = np.asarray(inputs['fc2_W']).astype(np.float16)
    fc2b = np.tile(np.asarray(inputs['fc2_b'])[None, :], (GPC, 1)).astype(np.float16)
    outW = np.asarray(inputs['out_W']).astype(np.float16)
    outb = np.tile(np.asarray(inputs['out_b']).reshape(1, 1), (GPC, 1)).astype(np.float32)

    maps = []
    for c in range(CORES):
        m = dict(
            idxA=pp['idxA'][c], idxB=pp['idxB'][c],
            dstl=pp['dstl'][c], pbin=pp['pbin'][c],
            iota=iota, ident=ident, onescol=onescol, brep3=brep3,
            pfcW=pfcW, pfcb=pfcb, fc1W=fc1W, fc1b=fc1b,
            fc2W=fc2W, fc2b=fc2b, outW=outW, outb=outb,
        )
        for b in range(2):
            lo, hi = pp['core_lo'][b][c], pp['core_hi'][b][c]
            xt = np.zeros((1024, NPC), np.float16)
            xt[:, 0:hi - lo] = xs[b][lo:hi].T.astype(np.float16)
            m[f'xT{b+1}'] = xt
            m[f'W{b+1}aug'] = wa[b]
        maps.append(m)
    return maps


_CACHE = {}


def _cache_key(pp):
    return (pp['NT'], pp['NSUP'],
            tuple(int(x) for x in pp['CHh'].reshape(-1)))


def get_nc(pp, **kw):
    key = (_cache_key(pp), tuple(sorted(kw.items())))
    if key not in _CACHE:
        _CACHE[key] = build_fused(pp, **kw)
    return _CACHE[key]


def kernel(**inputs):
    from concourse import bass_utils
    inputs = {k: np.asarray(v) for k, v in inputs.items()}
    pp = prep(inputs)
    nc = get_nc(pp)
    maps = host_inputs(inputs, pp)
    res = bass_utils.run_bass_kernel_spmd(nc, maps, core_ids=list(range(CORES)))
    return np.concatenate([res.results[c]['out'] for c in range(CORES)], axis=0)
